# revision 11
# baseline (speedup 1.0000x reference)
"""Trainium2 Bass kernel for a dense transformer block (B=2, T=2048, C=1024,
H=16, Dff=4096), SPMD across 8 NeuronCores.

Sharding: attention is head-parallel (2 heads/core); one AllToAll
redistributes the attention output into a token-parallel layout; projection,
layernorms and the FFN then run on each core's 512-token slice with full
weights. All on-device activations are kept feature-major (transposed) so
every matmul consumes weights exactly as stored; the host performs the
x -> x^T and out^T -> out transposes during input/output marshalling.
Matmuls run in float32r (fp32 storage, FP22 multiply, fp32 accumulate).
"""

import sys

sys.path.insert(0, "/opt/trn_rl_repo")

import numpy as np
import concourse.bacc as bacc
import concourse.mybir as mybir
import concourse.tile as tile
import concourse.bass_utils as bass_utils

try:  # make the NTFF profile shim importable as antenv.axon_hooks
    import antenv

    if "/opt/trn_rl_repo/antenv" not in antenv.__path__:
        antenv.__path__.append("/opt/trn_rl_repo/antenv")
except Exception:
    pass

f32 = mybir.dt.float32
f32r = mybir.dt.float32r
AF = mybir.ActivationFunctionType
ALU = mybir.AluOpType

NC = 8          # cores
B = 2           # batch
T = 2048        # sequence length
C = 1024        # model dim
H = 16          # heads
HD = 64         # head dim
HPC = H // NC   # heads per core (2)
DH = HPC * HD   # per-core head cols (128)
DFF = 4096
TOK = B * T     # 4096 tokens
TOKC = TOK // NC  # 512 tokens per core
CT = C // 128   # 8 c-tiles
FT = DFF // 128  # 32 ff-tiles
KT = T // 128   # 16 k-tiles per batch
QC = T // 512   # 4 q-chunks of 512 per batch
LN_EPS = 1e-5

_CACHE = {}


def _build(debug=False):
    nc = bacc.Bacc("TRN2", target_bir_lowering=False, debug=False, num_devices=NC)

    # ---- DRAM I/O (per-core values supplied via in_maps) ----
    xt_d = nc.dram_tensor("xt", [C, TOK], f32r, kind="ExternalInput")
    wq_d = nc.dram_tensor("wq_c", [128, CT, 128], f32r, kind="ExternalInput")
    wk_d = nc.dram_tensor("wk_c", [128, CT, 128], f32r, kind="ExternalInput")
    wv_d = nc.dram_tensor("wv_c", [128, CT, 128], f32r, kind="ExternalInput")
    xres_d = nc.dram_tensor("xres_c", [C, TOKC], f32r, kind="ExternalInput")
    wp_d = nc.dram_tensor("wproj", [C, C], f32r, kind="ExternalInput")
    w1_d = nc.dram_tensor("w1p", [FT, 128, CT, 128], f32r, kind="ExternalInput")
    w2_d = nc.dram_tensor("w2", [DFF, C], f32r, kind="ExternalInput")
    bias_d = nc.dram_tensor("biaspack", [128, 6 * CT], f32, kind="ExternalInput")
    b1_d = nc.dram_tensor("b1t", [128, FT], f32, kind="ExternalInput")
    ones_d = nc.dram_tensor("onesp", [128, 128], f32r, kind="ExternalInput")
    ident_d = nc.dram_tensor("ident", [128, 128], f32, kind="ExternalInput")
    mask_d = nc.dram_tensor("cmask", [128, 4, 512], f32r, kind="ExternalInput")
    out_d = nc.dram_tensor("out", [C, TOKC], f32, kind="ExternalOutput")
    if debug:
        dbg_attn = nc.dram_tensor("dbg_attn", [2, HD, TOK], f32, kind="ExternalOutput")
        dbg_qkv = nc.dram_tensor("dbg_qkv", [3, DH, TOK], f32, kind="ExternalOutput")

    with tile.TileContext(nc) as tc:
        with (
            nc.allow_low_precision(reason="float32r matmul inputs (~6e-5 rounding)"),
            tc.tile_pool(name="const", bufs=1) as p_const,
            tc.tile_pool(name="ln1p", bufs=CT) as p_ln1,
            tc.tile_pool(name="dram", bufs=1, space="DRAM") as p_dram,
        ):
            # ---- persistent constants ----
            ones = p_const.tile([128, 128], f32r, tag="ones")
            nc.sync.dma_start(ones[:], ones_d[:])
            biasp = p_const.tile([128, 6 * CT], f32, tag="biasp")
            nc.sync.dma_start(biasp[:], bias_d[:])
            b1t = p_const.tile([128, FT], f32, tag="b1t")
            nc.sync.dma_start(b1t[:], b1_d[:])
            # bias pack columns: [bproj | b2 | g1 | be1 | g2 | be2]
            bproj_b = biasp[:, 0 * CT:1 * CT]
            b2_b = biasp[:, 1 * CT:2 * CT]
            g1_b = biasp[:, 2 * CT:3 * CT]
            be1_b = biasp[:, 3 * CT:4 * CT]
            g2_b = biasp[:, 4 * CT:5 * CT]
            be2_b = biasp[:, 5 * CT:6 * CT]

            a2a_in = p_dram.tile([NC, DH, TOKC], f32r, tag="a2ai")
            a2a_out = p_dram.tile([NC, DH, TOKC], f32r, tag="a2ao")

            # ======== phase 1: QKV + attention (head-parallel) ========
            with (
                tc.tile_pool(name="attn", bufs=1) as p_attn,
                tc.tile_pool(name="p1c", bufs=1) as p1c,
                tc.tile_pool(name="xt", bufs=8) as p_xt,
                tc.tile_pool(name="qkv", bufs=1) as p_qkv,
                tc.tile_pool(name="es", bufs=3) as p_es,
                tc.tile_pool(name="small", bufs=2) as p_small,
                tc.tile_pool(name="ps1", bufs=6, space="PSUM") as ps1,
            ):
                # per-head attention outputs (feature-major rows 0-63)
                attnh = [
                    p_attn.tile([HD, TOK], f32r, tag=f"attn{h}", name=f"attnh{h}")
                    for h in range(HPC)
                ]
                ident = p1c.tile([128, 128], f32, tag="ident")
                nc.sync.dma_start(ident[:], ident_d[:])
                masks = p1c.tile([128, 4, 512], f32r, tag="masks")
                nc.sync.dma_start(masks[:], mask_d[:])
                wq_sb = p1c.tile([128, CT, 128], f32r, tag="wq")
                wk_sb = p1c.tile([128, CT, 128], f32r, tag="wk")
                wv_sb = p1c.tile([128, CT, 128], f32r, tag="wv")
                nc.sync.dma_start(wq_sb[:], wq_d[:])
                nc.sync.dma_start(wk_sb[:], wk_d[:])
                nc.sync.dma_start(wv_sb[:], wv_d[:])

                for b in range(B):
                    # ---- load x^T for this batch ----
                    xt_sb = []
                    for ct in range(CT):
                        t = p_xt.tile([128, T], f32r, tag="xt")
                        nc.sync.dma_start(
                            t[:], xt_d[ct * 128:(ct + 1) * 128, b * T:(b + 1) * T]
                        )
                        xt_sb.append(t)

                    qT = p_qkv.tile([DH, T], f32r, tag="q")
                    kT = p_qkv.tile([DH, T], f32r, tag="k")
                    vT = p_qkv.tile([DH, T], f32, tag="v")

                    # ---- QKV projections (feature-major) ----
                    for n in range(QC):
                        ncol = slice(n * 512, (n + 1) * 512)
                        pq = ps1.tile([128, 512], f32, tag="ps")
                        pk = ps1.tile([128, 512], f32, tag="ps")
                        pv = ps1.tile([128, 512], f32, tag="ps")
                        for ct in range(CT):
                            st = ct == 0
                            sp = ct == CT - 1
                            rhs = xt_sb[ct][:, ncol]
                            nc.tensor.matmul(
                                pq[:], wq_sb[:, ct, :], rhs, start=st, stop=sp
                            )
                            nc.tensor.matmul(
                                pk[:], wk_sb[:, ct, :], rhs, start=st, stop=sp
                            )
                            nc.tensor.matmul(
                                pv[:], wv_sb[:, ct, :], rhs, start=st, stop=sp
                            )
                        nc.vector.tensor_copy(qT[:, ncol], pq[:])
                        nc.vector.tensor_copy(kT[:, ncol], pk[:])
                        nc.scalar.copy(vT[:, ncol], pv[:])

                    # ---- V -> token-major; slot: [Vh0|ones|pad|Vh1|ones|pad] ----
                    SL = 132
                    vt = p_qkv.tile([128, KT, SL], f32r, tag="vt")
                    for kt in range(KT):
                        pt = ps1.tile([128, 128], f32, tag="ps")
                        nc.tensor.transpose(
                            pt[:], vT[:, kt * 128:(kt + 1) * 128], ident[:]
                        )
                        nc.vector.tensor_copy(vt[:, kt, 0:64], pt[:, 0:64])
                        nc.vector.tensor_copy(vt[:, kt, 66:130], pt[:, 64:128])
                        nc.vector.tensor_copy(vt[:, kt, 64:65], ones[:, 0:1])
                        nc.vector.tensor_copy(vt[:, kt, 130:131], ones[:, 1:2])

                    # ---- causal attention, per head / q-chunk ----
                    for h in range(HPC):
                        hrow = slice(h * 64, (h + 1) * 64)
                        for j in range(QC):
                            qs = qT[hrow, j * 512:(j + 1) * 512]
                            oacc = ps1.tile([65, 512], f32, tag="ps")
                            nkt = 4 * j + 4
                            for kt in range(nkt):
                                sps = ps1.tile([128, 512], f32, tag="ps")
                                nc.tensor.matmul(
                                    sps[:],
                                    kT[hrow, kt * 128:(kt + 1) * 128],
                                    qs,
                                    start=True, stop=True,
                                    tile_position=(64 * h, 0),
                                )
                                m = kt - 4 * j
                                es = p_es.tile([128, 512], f32r, tag="es")
                                if m < 0:
                                    nc.scalar.activation(
                                        es[:], sps[:], AF.Exp, scale=0.125
                                    )
                                else:
                                    esd = p_es.tile([128, 512], f32, tag="esd")
                                    nc.scalar.activation(
                                        esd[:], sps[:], AF.Exp, scale=0.125
                                    )
                                    nc.vector.tensor_mul(
                                        es[:], esd[:], masks[:, m, :].bitcast(f32)
                                    )
                                nc.tensor.matmul(
                                    oacc[:],
                                    vt[:, kt, 66 * h:66 * h + 65],
                                    es[:],
                                    start=(kt == 0), stop=(kt == nkt - 1),
                                )
                            # normalize: row 64 of oacc is sum(exp)
                            inv = p_small.tile([128, 512], f32r, tag="inv")
                            nc.vector.reciprocal(inv[64:65, :], oacc[64:65, :])
                            bps = ps1.tile([64, 512], f32, tag="ps")
                            nc.tensor.matmul(
                                bps[:], ones[64:65, 0:64], inv[64:65, :],
                                start=True, stop=True, tile_position=(64, 0),
                            )
                            bsb = p_small.tile([64, 512], f32, tag="bsb")
                            nc.scalar.copy(bsb[:], bps[:])
                            nc.vector.tensor_mul(
                                attnh[h][:, b * T + j * 512:b * T + (j + 1) * 512],
                                oacc[0:64, :],
                                bsb[:],
                            )

                    if debug:
                        nc.sync.dma_start(
                            dbg_qkv[0, :, b * T:(b + 1) * T], qT[:].bitcast(f32)
                        )
                        nc.sync.dma_start(
                            dbg_qkv[1, :, b * T:(b + 1) * T], kT[:].bitcast(f32)
                        )
                        nc.sync.dma_start(dbg_qkv[2, :, b * T:(b + 1) * T], vT[:])

                if debug:
                    for h in range(HPC):
                        nc.sync.dma_start(dbg_attn[h], attnh[h][:].bitcast(f32))

                # ---- AllToAll staging (inside phase 1: reads attnh) ----
                for j in range(NC):
                    for h in range(HPC):
                        nc.sync.dma_start(
                            a2a_in[j, h * 64:(h + 1) * 64, :],
                            attnh[h][:, j * TOKC:(j + 1) * TOKC],
                        )
                nc.gpsimd.collective_compute(
                    "AllToAll",
                    ALU.bypass,
                    replica_groups=[list(range(NC))],
                    ins=[a2a_in[:].opt()],
                    outs=[a2a_out[:].opt()],
                )

            # ======== phase 2: gather + proj + LN1 ========
            with (
                tc.tile_pool(name="agg", bufs=8) as p_agg,
                tc.tile_pool(name="wbig", bufs=8) as p_wbig,
                tc.tile_pool(name="act2", bufs=8) as p_act2,
                tc.tile_pool(name="tmp2", bufs=2) as p_tmp2,
                tc.tile_pool(name="ps2", bufs=6, space="PSUM") as ps2,
            ):
                ag = []
                for i in range(NC):
                    t = p_agg.tile([128, TOKC], f32r, tag="ag")
                    nc.sync.dma_start(t[:], a2a_out[i])
                    ag.append(t)

                wp_sb = []
                for kt in range(CT):
                    t = p_wbig.tile([128, C], f32r, tag="wp")
                    nc.sync.dma_start(t[:], wp_d[kt * 128:(kt + 1) * 128, :])
                    wp_sb.append(t)

                xres = []
                for ct in range(CT):
                    t = p_act2.tile([128, TOKC], f32r, tag="xres")
                    nc.sync.dma_start(t[:], xres_d[ct * 128:(ct + 1) * 128, :])
                    xres.append(t)

                def layer_norm(x_tiles, g_b, be_b, out_dtype, out_pool, out_tag,
                               sq_pool, tmp_pool, ps_pool):
                    """Feature-major LN across CT tiles of [128, TOKC]."""
                    s1 = ps_pool.tile([1, TOKC], f32, tag="ps")
                    s2 = ps_pool.tile([1, TOKC], f32, tag="ps")
                    for ct in range(CT):
                        nc.tensor.matmul(
                            s1[:], ones[:, 0:1], x_tiles[ct][:],
                            start=(ct == 0), stop=(ct == CT - 1),
                        )
                    for ct in range(CT):
                        sq = sq_pool.tile([128, TOKC], f32r, tag="sq")
                        nc.vector.tensor_mul(
                            sq[:],
                            x_tiles[ct][:].bitcast(f32),
                            x_tiles[ct][:].bitcast(f32),
                        )
                        nc.tensor.matmul(
                            s2[:], ones[:, 0:1], sq[:],
                            start=(ct == 0), stop=(ct == CT - 1),
                        )
                    nmu = tmp_pool.tile([1, TOKC], f32r, tag="nmu")
                    nc.vector.tensor_scalar_mul(nmu[:], s1[:], -1.0 / C)
                    ex2 = tmp_pool.tile([1, TOKC], f32, tag="ex2")
                    nc.vector.tensor_scalar_mul(ex2[:], s2[:], 1.0 / C)
                    mu2 = tmp_pool.tile([1, TOKC], f32, tag="mu2")
                    nc.vector.tensor_mul(
                        mu2[:], nmu[:].bitcast(f32), nmu[:].bitcast(f32)
                    )
                    var = tmp_pool.tile([1, TOKC], f32, tag="var")
                    nc.vector.tensor_sub(var[:], ex2[:], mu2[:])
                    nc.vector.tensor_scalar_add(var[:], var[:], LN_EPS)
                    sd = tmp_pool.tile([1, TOKC], f32, tag="sd")
                    nc.scalar.activation(sd[:], var[:], AF.Sqrt, bias=0.0)
                    rstd = tmp_pool.tile([1, TOKC], f32r, tag="rstd")
                    nc.vector.reciprocal(rstd[:], sd[:])
                    bmu = ps_pool.tile([128, TOKC], f32, tag="ps")
                    nc.tensor.matmul(
                        bmu[:], ones[0:1, :], nmu[:], start=True, stop=True
                    )
                    brs = ps_pool.tile([128, TOKC], f32, tag="ps")
                    nc.tensor.matmul(
                        brs[:], ones[0:1, :], rstd[:], start=True, stop=True
                    )
                    bmu_sb = tmp_pool.tile([128, TOKC], f32, tag="bmu")
                    nc.scalar.copy(bmu_sb[:], bmu[:])
                    brs_sb = tmp_pool.tile([128, TOKC], f32, tag="brs")
                    nc.scalar.copy(brs_sb[:], brs[:])
                    outs = []
                    for ct in range(CT):
                        t1 = tmp_pool.tile([128, TOKC], f32, tag="lntmp")
                        nc.vector.tensor_add(
                            t1[:], x_tiles[ct][:].bitcast(f32), bmu_sb[:]
                        )
                        t2 = tmp_pool.tile([128, TOKC], f32, tag="lntmp2")
                        nc.vector.tensor_mul(t2[:], t1[:], brs_sb[:])
                        o = out_pool.tile([128, TOKC], out_dtype, tag=out_tag)
                        nc.scalar.activation(
                            o[:], t2[:], AF.Identity,
                            bias=be_b[:, ct:ct + 1], scale=g_b[:, ct:ct + 1],
                        )
                        outs.append(o)
                    return outs

                # ---- proj + bias + residual ----
                x1 = []
                for mt in range(CT):
                    yps = ps2.tile([128, TOKC], f32, tag="ps")
                    for kt in range(CT):
                        nc.tensor.matmul(
                            yps[:], wp_sb[kt][:, mt * 128:(mt + 1) * 128], ag[kt][:],
                            start=(kt == 0), stop=(kt == CT - 1),
                        )
                    t1 = p_tmp2.tile([128, TOKC], f32, tag="projt")
                    nc.scalar.activation(
                        t1[:], yps[:], AF.Identity, bias=bproj_b[:, mt:mt + 1]
                    )
                    xr = p_act2.tile([128, TOKC], f32r, tag="x1")
                    nc.vector.tensor_add(xr[:], t1[:], xres[mt][:].bitcast(f32))
                    x1.append(xr)

                ln1 = layer_norm(
                    x1, g1_b, be1_b, f32r, p_ln1, "ln1", p_tmp2, p_tmp2, ps2
                )

            # ======== phase 3: FFN + LN2 + out ========
            with (
                tc.tile_pool(name="hff", bufs=FT) as p_hff,
                tc.tile_pool(name="w1s", bufs=4) as p_w1,
                tc.tile_pool(name="w2s", bufs=4) as p_w2,
                tc.tile_pool(name="act3", bufs=8) as p_act3,
                tc.tile_pool(name="tmp3", bufs=2) as p_tmp3,
                tc.tile_pool(name="outp", bufs=2) as p_out,
            ):
                # ---- FFN1: h = relu(w1^T @ ln1 + b1) ----
                hff = []
                with tc.tile_pool(name="ps3a", bufs=4, space="PSUM") as ps3a:
                    for mt in range(FT):
                        w1t = p_w1.tile([128, CT, 128], f32r, tag="w1")
                        nc.sync.dma_start(w1t[:], w1_d[mt])
                        yps = ps3a.tile([128, TOKC], f32, tag="ps")
                        for kt in range(CT):
                            nc.tensor.matmul(
                                yps[:], w1t[:, kt, :], ln1[kt][:],
                                start=(kt == 0), stop=(kt == CT - 1),
                            )
                        hf = p_hff.tile([128, TOKC], f32r, tag="hff")
                        nc.scalar.activation(
                            hf[:], yps[:], AF.Relu, bias=b1t[:, mt:mt + 1]
                        )
                        hff.append(hf)

                # ---- FFN2 (kt-outer, 8 accumulators) + LN2 ----
                with tc.tile_pool(name="ps3b", bufs=8, space="PSUM") as ps3b:
                    accs = [
                        ps3b.tile([128, TOKC], f32, tag="ps", name=f"acc{mt}")
                        for mt in range(CT)
                    ]
                    for kt in range(FT):
                        w2t = p_w2.tile([128, C], f32r, tag="w2")
                        nc.sync.dma_start(w2t[:], w2_d[kt * 128:(kt + 1) * 128, :])
                        for mt in range(CT):
                            nc.tensor.matmul(
                                accs[mt][:],
                                w2t[:, mt * 128:(mt + 1) * 128],
                                hff[kt][:],
                                start=(kt == 0), stop=(kt == FT - 1),
                            )
                    x2 = []
                    for mt in range(CT):
                        t1 = p_tmp3.tile([128, TOKC], f32, tag="ffn2t")
                        nc.scalar.activation(
                            t1[:], accs[mt][:], AF.Identity, bias=b2_b[:, mt:mt + 1]
                        )
                        xr = p_act3.tile([128, TOKC], f32r, tag="x2")
                        nc.vector.tensor_add(xr[:], t1[:], ln1[mt][:].bitcast(f32))
                        x2.append(xr)

                    out_tiles = layer_norm(
                        x2, g2_b, be2_b, f32, p_out, "outt", p_tmp3, p_tmp3, ps3b
                    )
                    for ct in range(CT):
                        nc.sync.dma_start(
                            out_d[ct * 128:(ct + 1) * 128, :], out_tiles[ct][:]
                        )

    nc.compile()
    return nc


def _pack_inputs(inputs):
    """Host-side sharding/marshalling. Returns in_maps for the 8 cores."""
    x = np.asarray(inputs["x"], dtype=np.float32)
    xf = np.ascontiguousarray(x.reshape(TOK, C))
    xt = np.ascontiguousarray(xf.T)  # [C, TOK]
    wq = np.asarray(inputs["wq"], dtype=np.float32)
    wk = np.asarray(inputs["wk"], dtype=np.float32)
    wv = np.asarray(inputs["wv"], dtype=np.float32)
    wproj = np.ascontiguousarray(np.asarray(inputs["w_proj"], dtype=np.float32))
    w1 = np.asarray(inputs["w1"], dtype=np.float32)
    w2 = np.ascontiguousarray(np.asarray(inputs["w2"], dtype=np.float32))
    # w1 packed per ff-tile: [FT, 128(p), CT, 128(f)];  w1 is [C, DFF]
    w1p = np.ascontiguousarray(
        w1.reshape(CT, 128, FT, 128).transpose(2, 1, 0, 3)
    )

    def tile_vec(v, n):
        return np.ascontiguousarray(
            np.asarray(v, dtype=np.float32).reshape(n, 128).T
        )

    biaspack = np.zeros((128, 6 * CT), dtype=np.float32)
    biaspack[:, 0 * CT:1 * CT] = tile_vec(inputs["b_proj"], CT)
    biaspack[:, 1 * CT:2 * CT] = tile_vec(inputs["b2"], CT)
    biaspack[:, 2 * CT:3 * CT] = tile_vec(inputs["g1"], CT)
    biaspack[:, 3 * CT:4 * CT] = tile_vec(inputs["be1"], CT)
    biaspack[:, 4 * CT:5 * CT] = tile_vec(inputs["g2"], CT)
    biaspack[:, 5 * CT:6 * CT] = tile_vec(inputs["be2"], CT)
    b1t = tile_vec(inputs["b1"], FT)

    # causal masks for the 4 diagonal offsets, packed [128, 4, 512]
    r = np.arange(128)[:, None]
    ccol = np.arange(512)[None, :]
    cmask = np.stack(
        [(ccol >= r + 128 * m).astype(np.float32) for m in range(4)], axis=1
    )
    cmask = np.ascontiguousarray(cmask)  # [128, 4, 512]
    onesp = np.ones((128, 128), dtype=np.float32)
    ident = np.eye(128, dtype=np.float32)

    in_maps = []
    for c in range(NC):
        hcol = slice(c * DH, (c + 1) * DH)

        def pack_w(w):
            return np.ascontiguousarray(
                w[:, hcol].reshape(CT, 128, DH).transpose(1, 0, 2)
            )

        in_maps.append(
            {
                "xt": xt,
                "wq_c": pack_w(wq),
                "wk_c": pack_w(wk),
                "wv_c": pack_w(wv),
                "xres_c": np.ascontiguousarray(
                    xt[:, c * TOKC:(c + 1) * TOKC]
                ),
                "wproj": wproj,
                "w1p": w1p,
                "w2": w2,
                "biaspack": biaspack,
                "b1t": b1t,
                "onesp": onesp,
                "ident": ident,
                "cmask": cmask,
            }
        )
    return in_maps


def _run(inputs, trace=False, debug=False):
    key = "dbg" if debug else "rel"
    if key not in _CACHE:
        _CACHE[key] = _build(debug=debug)
    nc = _CACHE[key]
    in_maps = _pack_inputs(inputs)
    res = bass_utils.run_bass_kernel_spmd(
        nc, in_maps, core_ids=list(range(NC)), trace=trace
    )
    out = np.empty((TOK, C), dtype=np.float32)
    for c in range(NC):
        out[c * TOKC:(c + 1) * TOKC, :] = res.results[c]["out"].T
    return out.reshape(B, T, C), res


def kernel(**inputs) -> np.ndarray:
    out, _ = _run(inputs, trace=False, debug=False)
    return out


# revision 16
# speedup vs baseline: 1.1546x; 1.1546x over previous
"""Trainium2 Bass kernel for a dense transformer block (B=2, T=2048, C=1024,
H=16, Dff=4096), SPMD across 8 NeuronCores.

Sharding: attention is head-parallel (2 heads/core); one AllToAll
redistributes the attention output into a token-parallel layout; projection,
layernorms and the FFN then run on each core's 512-token slice with full
weights. All on-device activations are kept feature-major (transposed) so
every matmul consumes weights exactly as stored; the host performs the
x -> x^T and out^T -> out transposes during input/output marshalling.
Matmuls run in float32r (fp32 storage, FP22 multiply, fp32 accumulate).
"""

import sys

sys.path.insert(0, "/opt/trn_rl_repo")

import numpy as np
import concourse.bacc as bacc
import concourse.mybir as mybir
import concourse.tile as tile
import concourse.bass_utils as bass_utils

try:  # make the NTFF profile shim importable as antenv.axon_hooks
    import antenv

    if "/opt/trn_rl_repo/antenv" not in antenv.__path__:
        antenv.__path__.append("/opt/trn_rl_repo/antenv")
except Exception:
    pass

f32 = mybir.dt.float32
f32r = mybir.dt.float32r
AF = mybir.ActivationFunctionType
ALU = mybir.AluOpType

NC = 8          # cores
B = 2           # batch
T = 2048        # sequence length
C = 1024        # model dim
H = 16          # heads
HD = 64         # head dim
HPC = H // NC   # heads per core (2)
DH = HPC * HD   # per-core head cols (128)
DFF = 4096
TOK = B * T     # 4096 tokens
TOKC = TOK // NC  # 512 tokens per core
CT = C // 128   # 8 c-tiles
FT = DFF // 128  # 32 ff-tiles
KT = T // 128   # 16 k-tiles per batch
QC = T // 512   # 4 q-chunks of 512 per batch
LN_EPS = 1e-5

_CACHE = {}


def _build(debug=False):
    nc = bacc.Bacc("TRN2", target_bir_lowering=False, debug=False, num_devices=NC)

    # ---- DRAM I/O (per-core values supplied via in_maps) ----
    xt_d = nc.dram_tensor("xt", [C, TOK], f32r, kind="ExternalInput")
    wq_d = nc.dram_tensor("wq_c", [128, CT, 128], f32r, kind="ExternalInput")
    wk_d = nc.dram_tensor("wk_c", [128, CT, 128], f32r, kind="ExternalInput")
    wv_d = nc.dram_tensor("wv_c", [128, CT, 128], f32r, kind="ExternalInput")
    xres_d = nc.dram_tensor("xres_c", [C, TOKC], f32r, kind="ExternalInput")
    wp_d = nc.dram_tensor("wproj", [C, C], f32r, kind="ExternalInput")
    w1_d = nc.dram_tensor("w1p", [FT, 128, CT, 128], f32r, kind="ExternalInput")
    w2_d = nc.dram_tensor("w2", [DFF, C], f32r, kind="ExternalInput")
    bias_d = nc.dram_tensor("biaspack", [128, 6 * CT], f32, kind="ExternalInput")
    b1_d = nc.dram_tensor("b1t", [128, FT], f32, kind="ExternalInput")
    ones_d = nc.dram_tensor("onesp", [128, 128], f32r, kind="ExternalInput")
    ident_d = nc.dram_tensor("ident", [128, 128], f32, kind="ExternalInput")
    mask_d = nc.dram_tensor("cmask", [128, 4, 512], f32r, kind="ExternalInput")
    out_d = nc.dram_tensor("out", [C, TOKC], f32, kind="ExternalOutput")
    if debug:
        dbg_attn = nc.dram_tensor("dbg_attn", [2, HD, TOK], f32, kind="ExternalOutput")
        dbg_qkv = nc.dram_tensor("dbg_qkv", [3, DH, TOK], f32, kind="ExternalOutput")

    with tile.TileContext(nc) as tc:
        with (
            nc.allow_low_precision(reason="float32r matmul inputs (~6e-5 rounding)"),
            tc.tile_pool(name="const", bufs=1) as p_const,
            tc.tile_pool(name="ln1p", bufs=CT) as p_ln1,
            tc.tile_pool(name="dram", bufs=1, space="DRAM") as p_dram,
        ):
            # ---- persistent constants ----
            ones = p_const.tile([128, 128], f32r, tag="ones")
            nc.sync.dma_start(ones[:], ones_d[:])
            biasp = p_const.tile([128, 6 * CT], f32, tag="biasp")
            nc.sync.dma_start(biasp[:], bias_d[:])
            b1t = p_const.tile([128, FT], f32, tag="b1t")
            nc.sync.dma_start(b1t[:], b1_d[:])
            # bias pack columns: [bproj | b2 | g1 | be1 | g2 | be2]
            bproj_b = biasp[:, 0 * CT:1 * CT]
            b2_b = biasp[:, 1 * CT:2 * CT]
            g1_b = biasp[:, 2 * CT:3 * CT]
            be1_b = biasp[:, 3 * CT:4 * CT]
            g2_b = biasp[:, 4 * CT:5 * CT]
            be2_b = biasp[:, 5 * CT:6 * CT]

            a2a_in = p_dram.tile([NC, DH, TOKC], f32r, tag="a2ai")
            a2a_out = p_dram.tile([NC, DH, TOKC], f32r, tag="a2ao")

            # ======== phase 1: QKV + attention (head-parallel) ========
            with (
                tc.tile_pool(name="attn", bufs=1) as p_attn,
                tc.tile_pool(name="p1c", bufs=1) as p1c,
                tc.tile_pool(name="xt", bufs=8) as p_xt,
                tc.tile_pool(name="qkv", bufs=1) as p_qkv,
                tc.tile_pool(name="es", bufs=3) as p_es,
                tc.tile_pool(name="small", bufs=2) as p_small,
                tc.tile_pool(name="ps1", bufs=6, space="PSUM") as ps1,
            ):
                # per-head attention outputs (feature-major rows 0-63)
                attnh = [
                    p_attn.tile([HD, TOK], f32r, tag=f"attn{h}", name=f"attnh{h}")
                    for h in range(HPC)
                ]
                ident = p1c.tile([128, 128], f32, tag="ident")
                nc.sync.dma_start(ident[:], ident_d[:])
                masks = p1c.tile([128, 4, 512], f32r, tag="masks")
                nc.sync.dma_start(masks[:], mask_d[:])
                wq_sb = p1c.tile([128, CT, 128], f32r, tag="wq")
                wk_sb = p1c.tile([128, CT, 128], f32r, tag="wk")
                wv_sb = p1c.tile([128, CT, 128], f32r, tag="wv")
                nc.sync.dma_start(wq_sb[:], wq_d[:])
                nc.sync.dma_start(wk_sb[:], wk_d[:])
                nc.sync.dma_start(wv_sb[:], wv_d[:])

                for b in range(B):
                    qT = p_qkv.tile([DH, T], f32r, tag="q")
                    kT = p_qkv.tile([DH, T], f32r, tag="k")
                    vT = p_qkv.tile([DH, T], f32, tag="v")

                    # ---- QKV projections (feature-major), x^T streamed per
                    # (ct, chunk) so only ~2 chunks of x^T are resident ----
                    for n in range(QC):
                        ncol = slice(n * 512, (n + 1) * 512)
                        xt_sb = []
                        for ct in range(CT):
                            t = p_xt.tile([128, 512], f32r, tag="xt", bufs=16)
                            nc.sync.dma_start(
                                t[:],
                                xt_d[ct * 128:(ct + 1) * 128,
                                     b * T + n * 512:b * T + (n + 1) * 512],
                            )
                            xt_sb.append(t)
                        pq = ps1.tile([128, 512], f32, tag="sps", bufs=5)
                        pk = ps1.tile([128, 512], f32, tag="sps", bufs=5)
                        for ct in range(CT):
                            st = ct == 0
                            sp = ct == CT - 1
                            nc.tensor.matmul(
                                pq[:], wq_sb[:, ct, :], xt_sb[ct][:],
                                start=st, stop=sp,
                            )
                            nc.tensor.matmul(
                                pk[:], wk_sb[:, ct, :], xt_sb[ct][:],
                                start=st, stop=sp,
                            )
                        nc.vector.tensor_copy(qT[:, ncol], pq[:])
                        nc.vector.tensor_copy(kT[:, ncol], pk[:])
                        pv = ps1.tile([128, 512], f32, tag="sps", bufs=5)
                        for ct in range(CT):
                            nc.tensor.matmul(
                                pv[:], wv_sb[:, ct, :], xt_sb[ct][:],
                                start=(ct == 0), stop=(ct == CT - 1),
                            )
                        nc.scalar.copy(vT[:, ncol], pv[:])

                    # ---- V -> token-major; slot: [Vh0|ones|pad|Vh1|ones|pad] ----
                    SL = 132
                    vt = p_qkv.tile([128, KT, SL], f32r, tag="vt")
                    for kt in range(KT):
                        pt = ps1.tile([128, 128], f32, tag="misc", bufs=1)
                        nc.tensor.transpose(
                            pt[:], vT[:, kt * 128:(kt + 1) * 128], ident[:]
                        )
                        nc.vector.tensor_copy(vt[:, kt, 0:64], pt[:, 0:64])
                        nc.vector.tensor_copy(vt[:, kt, 66:130], pt[:, 64:128])
                        nc.vector.tensor_copy(vt[:, kt, 64:65], ones[:, 0:1])
                        nc.vector.tensor_copy(vt[:, kt, 130:131], ones[:, 1:2])

                    # ---- causal attention: both heads interleaved per k-tile
                    # (adjacent score matmuls pack into disjoint array rows) ----
                    for j in range(QC):
                        nkt = 4 * j + 4
                        oacc = [
                            ps1.tile([65, 512], f32, tag="oacc", bufs=2,
                                     name=f"oacc{h}")
                            for h in range(HPC)
                        ]
                        for kt in range(nkt):
                            m = kt - 4 * j
                            es2 = []
                            for h in range(HPC):
                                hrow = slice(h * 64, (h + 1) * 64)
                                sps = ps1.tile([128, 512], f32, tag="sps", bufs=5)
                                nc.tensor.matmul(
                                    sps[:],
                                    kT[hrow, kt * 128:(kt + 1) * 128],
                                    qT[hrow, j * 512:(j + 1) * 512],
                                    start=True, stop=True,
                                    tile_position=(64 * h, 0),
                                )
                                es = p_es.tile([128, 512], f32r, tag="es", bufs=6)
                                if m < 0:
                                    nc.scalar.activation(
                                        es[:], sps[:], AF.Exp, scale=0.125
                                    )
                                else:
                                    esd = p_es.tile([128, 512], f32, tag="esd",
                                                    bufs=2)
                                    nc.scalar.activation(
                                        esd[:], sps[:], AF.Exp, scale=0.125
                                    )
                                    nc.vector.tensor_mul(
                                        es[:], esd[:], masks[:, m, :].bitcast(f32)
                                    )
                                es2.append(es)
                            for h in range(HPC):
                                nc.tensor.matmul(
                                    oacc[h][:],
                                    vt[:, kt, 66 * h:66 * h + 65],
                                    es2[h][:],
                                    start=(kt == 0), stop=(kt == nkt - 1),
                                )
                        for h in range(HPC):
                            # free the PSUM accum early via two same-base copies
                            osb = p_small.tile([64, 512], f32, tag="osb", bufs=4)
                            nc.scalar.copy(osb[:], oacc[h][0:64, :])
                            sr = p_small.tile([128, 512], f32r, tag="sr")
                            nc.vector.tensor_copy(sr[64:65, :], oacc[h][64:65, :])
                            # broadcast sums across partitions, then approx-recip
                            bps = ps1.tile([64, 512], f32, tag="misc", bufs=1)
                            nc.tensor.matmul(
                                bps[:], ones[64:65, 0:64], sr[64:65, :],
                                start=True, stop=True, tile_position=(64, 0),
                            )
                            ibc = p_small.tile([64, 512], f32, tag="ibc")
                            nc.vector.reciprocal_approx_fast(ibc[:], bps[:])
                            nc.vector.tensor_mul(
                                attnh[h][:, b * T + j * 512:b * T + (j + 1) * 512],
                                osb[:],
                                ibc[:],
                            )

                    if debug:
                        nc.sync.dma_start(
                            dbg_qkv[0, :, b * T:(b + 1) * T], qT[:].bitcast(f32)
                        )
                        nc.sync.dma_start(
                            dbg_qkv[1, :, b * T:(b + 1) * T], kT[:].bitcast(f32)
                        )
                        nc.sync.dma_start(dbg_qkv[2, :, b * T:(b + 1) * T], vT[:])

                if debug:
                    for h in range(HPC):
                        nc.sync.dma_start(dbg_attn[h], attnh[h][:].bitcast(f32))

                # ---- AllToAll staging (inside phase 1: reads attnh) ----
                for j in range(NC):
                    for h in range(HPC):
                        nc.sync.dma_start(
                            a2a_in[j, h * 64:(h + 1) * 64, :],
                            attnh[h][:, j * TOKC:(j + 1) * TOKC],
                        )
                nc.gpsimd.collective_compute(
                    "AllToAll",
                    ALU.bypass,
                    replica_groups=[list(range(NC))],
                    ins=[a2a_in[:].opt()],
                    outs=[a2a_out[:].opt()],
                )

            # ======== phase 2: gather + proj + LN1 ========
            with (
                tc.tile_pool(name="agg", bufs=8) as p_agg,
                tc.tile_pool(name="wbig", bufs=8) as p_wbig,
                tc.tile_pool(name="act2", bufs=8) as p_act2,
                tc.tile_pool(name="tmp2", bufs=2) as p_tmp2,
                tc.tile_pool(name="ps2", bufs=6, space="PSUM") as ps2,
            ):
                ag = []
                for i in range(NC):
                    t = p_agg.tile([128, TOKC], f32r, tag="ag")
                    nc.sync.dma_start(t[:], a2a_out[i])
                    ag.append(t)

                wp_sb = []
                for kt in range(CT):
                    t = p_wbig.tile([128, C], f32r, tag="wp")
                    nc.sync.dma_start(t[:], wp_d[kt * 128:(kt + 1) * 128, :])
                    wp_sb.append(t)

                xres = []
                for ct in range(CT):
                    t = p_act2.tile([128, TOKC], f32r, tag="xres")
                    nc.sync.dma_start(t[:], xres_d[ct * 128:(ct + 1) * 128, :])
                    xres.append(t)

                def layer_norm(x_tiles, g_b, be_b, out_dtype, out_pool, out_tag,
                               sq_pool, tmp_pool, ps_pool):
                    """Feature-major LN across CT tiles of [128, TOKC]."""
                    s1 = ps_pool.tile([1, TOKC], f32, tag="ps")
                    s2 = ps_pool.tile([1, TOKC], f32, tag="ps")
                    for ct in range(CT):
                        nc.tensor.matmul(
                            s1[:], ones[:, 0:1], x_tiles[ct][:],
                            start=(ct == 0), stop=(ct == CT - 1),
                        )
                    for ct in range(CT):
                        sq = sq_pool.tile([128, TOKC], f32r, tag="sq")
                        nc.vector.tensor_mul(
                            sq[:],
                            x_tiles[ct][:].bitcast(f32),
                            x_tiles[ct][:].bitcast(f32),
                        )
                        nc.tensor.matmul(
                            s2[:], ones[:, 0:1], sq[:],
                            start=(ct == 0), stop=(ct == CT - 1),
                        )
                    nmu = tmp_pool.tile([1, TOKC], f32r, tag="nmu")
                    nc.vector.tensor_scalar_mul(nmu[:], s1[:], -1.0 / C)
                    ex2 = tmp_pool.tile([1, TOKC], f32, tag="ex2")
                    nc.vector.tensor_scalar_mul(ex2[:], s2[:], 1.0 / C)
                    mu2 = tmp_pool.tile([1, TOKC], f32, tag="mu2")
                    nc.vector.tensor_mul(
                        mu2[:], nmu[:].bitcast(f32), nmu[:].bitcast(f32)
                    )
                    var = tmp_pool.tile([1, TOKC], f32, tag="var")
                    nc.vector.tensor_sub(var[:], ex2[:], mu2[:])
                    nc.vector.tensor_scalar_add(var[:], var[:], LN_EPS)
                    sd = tmp_pool.tile([1, TOKC], f32r, tag="sd")
                    nc.scalar.activation(sd[:], var[:], AF.Sqrt, bias=0.0)
                    bmu = ps_pool.tile([128, TOKC], f32, tag="ps")
                    nc.tensor.matmul(
                        bmu[:], ones[0:1, :], nmu[:], start=True, stop=True
                    )
                    brs = ps_pool.tile([128, TOKC], f32, tag="ps")
                    nc.tensor.matmul(
                        brs[:], ones[0:1, :], sd[:], start=True, stop=True
                    )
                    bmu_sb = tmp_pool.tile([128, TOKC], f32, tag="bmu")
                    nc.scalar.copy(bmu_sb[:], bmu[:])
                    brs_sb = tmp_pool.tile([128, TOKC], f32, tag="brs")
                    nc.vector.reciprocal_approx_fast(brs_sb[:], brs[:])
                    outs = []
                    for ct in range(CT):
                        t1 = tmp_pool.tile([128, TOKC], f32, tag="lntmp")
                        nc.vector.tensor_add(
                            t1[:], x_tiles[ct][:].bitcast(f32), bmu_sb[:]
                        )
                        t2 = tmp_pool.tile([128, TOKC], f32, tag="lntmp2")
                        nc.vector.tensor_mul(t2[:], t1[:], brs_sb[:])
                        o = out_pool.tile([128, TOKC], out_dtype, tag=out_tag)
                        nc.scalar.activation(
                            o[:], t2[:], AF.Identity,
                            bias=be_b[:, ct:ct + 1], scale=g_b[:, ct:ct + 1],
                        )
                        outs.append(o)
                    return outs

                # ---- proj + bias + residual ----
                x1 = []
                for mt in range(CT):
                    yps = ps2.tile([128, TOKC], f32, tag="ps")
                    for kt in range(CT):
                        nc.tensor.matmul(
                            yps[:], wp_sb[kt][:, mt * 128:(mt + 1) * 128], ag[kt][:],
                            start=(kt == 0), stop=(kt == CT - 1),
                        )
                    t1 = p_tmp2.tile([128, TOKC], f32, tag="projt")
                    nc.scalar.activation(
                        t1[:], yps[:], AF.Identity, bias=bproj_b[:, mt:mt + 1]
                    )
                    xr = p_act2.tile([128, TOKC], f32r, tag="x1")
                    nc.vector.tensor_add(xr[:], t1[:], xres[mt][:].bitcast(f32))
                    x1.append(xr)

                ln1 = layer_norm(
                    x1, g1_b, be1_b, f32r, p_ln1, "ln1", p_tmp2, p_tmp2, ps2
                )

            # ======== phase 3: FFN + LN2 + out ========
            with (
                tc.tile_pool(name="hff", bufs=FT) as p_hff,
                tc.tile_pool(name="w1s", bufs=4) as p_w1,
                tc.tile_pool(name="w2s", bufs=4) as p_w2,
                tc.tile_pool(name="act3", bufs=8) as p_act3,
                tc.tile_pool(name="tmp3", bufs=2) as p_tmp3,
                tc.tile_pool(name="outp", bufs=2) as p_out,
            ):
                # ---- FFN1: h = relu(w1^T @ ln1 + b1) ----
                hff = []
                with tc.tile_pool(name="ps3a", bufs=4, space="PSUM") as ps3a:
                    for mt in range(FT):
                        w1t = p_w1.tile([128, CT, 128], f32r, tag="w1")
                        nc.sync.dma_start(w1t[:], w1_d[mt])
                        yps = ps3a.tile([128, TOKC], f32, tag="ps")
                        for kt in range(CT):
                            nc.tensor.matmul(
                                yps[:], w1t[:, kt, :], ln1[kt][:],
                                start=(kt == 0), stop=(kt == CT - 1),
                            )
                        hf = p_hff.tile([128, TOKC], f32r, tag="hff")
                        nc.scalar.activation(
                            hf[:], yps[:], AF.Relu, bias=b1t[:, mt:mt + 1]
                        )
                        hff.append(hf)

                # ---- FFN2 (kt-outer, 8 accumulators) + LN2 ----
                with tc.tile_pool(name="ps3b", bufs=8, space="PSUM") as ps3b:
                    accs = [
                        ps3b.tile([128, TOKC], f32, tag="ps", name=f"acc{mt}")
                        for mt in range(CT)
                    ]
                    for kt in range(FT):
                        w2t = p_w2.tile([128, C], f32r, tag="w2")
                        nc.sync.dma_start(w2t[:], w2_d[kt * 128:(kt + 1) * 128, :])
                        for mt in range(CT):
                            nc.tensor.matmul(
                                accs[mt][:],
                                w2t[:, mt * 128:(mt + 1) * 128],
                                hff[kt][:],
                                start=(kt == 0), stop=(kt == FT - 1),
                            )
                    x2 = []
                    for mt in range(CT):
                        t1 = p_tmp3.tile([128, TOKC], f32, tag="ffn2t")
                        nc.scalar.activation(
                            t1[:], accs[mt][:], AF.Identity, bias=b2_b[:, mt:mt + 1]
                        )
                        xr = p_act3.tile([128, TOKC], f32r, tag="x2")
                        nc.vector.tensor_add(xr[:], t1[:], ln1[mt][:].bitcast(f32))
                        x2.append(xr)

                    out_tiles = layer_norm(
                        x2, g2_b, be2_b, f32, p_out, "outt", p_tmp3, p_tmp3, ps3b
                    )
                    for ct in range(CT):
                        nc.sync.dma_start(
                            out_d[ct * 128:(ct + 1) * 128, :], out_tiles[ct][:]
                        )

    nc.compile()
    return nc


def _pack_inputs(inputs):
    """Host-side sharding/marshalling. Returns in_maps for the 8 cores."""
    x = np.asarray(inputs["x"], dtype=np.float32)
    xf = np.ascontiguousarray(x.reshape(TOK, C))
    xt = np.ascontiguousarray(xf.T)  # [C, TOK]
    wq = np.asarray(inputs["wq"], dtype=np.float32)
    wk = np.asarray(inputs["wk"], dtype=np.float32)
    wv = np.asarray(inputs["wv"], dtype=np.float32)
    wproj = np.ascontiguousarray(np.asarray(inputs["w_proj"], dtype=np.float32))
    w1 = np.asarray(inputs["w1"], dtype=np.float32)
    w2 = np.ascontiguousarray(np.asarray(inputs["w2"], dtype=np.float32))
    # w1 packed per ff-tile: [FT, 128(p), CT, 128(f)];  w1 is [C, DFF]
    w1p = np.ascontiguousarray(
        w1.reshape(CT, 128, FT, 128).transpose(2, 1, 0, 3)
    )

    def tile_vec(v, n):
        return np.ascontiguousarray(
            np.asarray(v, dtype=np.float32).reshape(n, 128).T
        )

    biaspack = np.zeros((128, 6 * CT), dtype=np.float32)
    biaspack[:, 0 * CT:1 * CT] = tile_vec(inputs["b_proj"], CT)
    biaspack[:, 1 * CT:2 * CT] = tile_vec(inputs["b2"], CT)
    biaspack[:, 2 * CT:3 * CT] = tile_vec(inputs["g1"], CT)
    biaspack[:, 3 * CT:4 * CT] = tile_vec(inputs["be1"], CT)
    biaspack[:, 4 * CT:5 * CT] = tile_vec(inputs["g2"], CT)
    biaspack[:, 5 * CT:6 * CT] = tile_vec(inputs["be2"], CT)
    b1t = tile_vec(inputs["b1"], FT)

    # causal masks for the 4 diagonal offsets, packed [128, 4, 512]
    r = np.arange(128)[:, None]
    ccol = np.arange(512)[None, :]
    cmask = np.stack(
        [(ccol >= r + 128 * m).astype(np.float32) for m in range(4)], axis=1
    )
    cmask = np.ascontiguousarray(cmask)  # [128, 4, 512]
    onesp = np.ones((128, 128), dtype=np.float32)
    ident = np.eye(128, dtype=np.float32)

    in_maps = []
    for c in range(NC):
        hcol = slice(c * DH, (c + 1) * DH)

        def pack_w(w):
            return np.ascontiguousarray(
                w[:, hcol].reshape(CT, 128, DH).transpose(1, 0, 2)
            )

        in_maps.append(
            {
                "xt": xt,
                "wq_c": pack_w(wq),
                "wk_c": pack_w(wk),
                "wv_c": pack_w(wv),
                "xres_c": np.ascontiguousarray(
                    xt[:, c * TOKC:(c + 1) * TOKC]
                ),
                "wproj": wproj,
                "w1p": w1p,
                "w2": w2,
                "biaspack": biaspack,
                "b1t": b1t,
                "onesp": onesp,
                "ident": ident,
                "cmask": cmask,
            }
        )
    return in_maps


def _run(inputs, trace=False, debug=False):
    key = "dbg" if debug else "rel"
    if key not in _CACHE:
        _CACHE[key] = _build(debug=debug)
    nc = _CACHE[key]
    in_maps = _pack_inputs(inputs)
    res = bass_utils.run_bass_kernel_spmd(
        nc, in_maps, core_ids=list(range(NC)), trace=trace
    )
    out = np.empty((TOK, C), dtype=np.float32)
    for c in range(NC):
        out[c * TOKC:(c + 1) * TOKC, :] = res.results[c]["out"].T
    return out.reshape(B, T, C), res


def kernel(**inputs) -> np.ndarray:
    out, _ = _run(inputs, trace=False, debug=False)
    return out


# revision 19
# speedup vs baseline: 1.2301x; 1.0653x over previous
"""Trainium2 Bass kernel for a dense transformer block (B=2, T=2048, C=1024,
H=16, Dff=4096), SPMD across 8 NeuronCores.

Sharding: attention is head-parallel (2 heads/core); one AllToAll
redistributes the attention output into a token-parallel layout; projection,
layernorms and the FFN then run on each core's 512-token slice with full
weights. All on-device activations are kept feature-major (transposed) so
every matmul consumes weights exactly as stored; the host performs the
x -> x^T and out^T -> out transposes during input/output marshalling.
Matmuls run in float32r (fp32 storage, FP22 multiply, fp32 accumulate).
"""

import sys

sys.path.insert(0, "/opt/trn_rl_repo")

import numpy as np
import concourse.bacc as bacc
import concourse.mybir as mybir
import concourse.tile as tile
import concourse.bass_utils as bass_utils

try:  # make the NTFF profile shim importable as antenv.axon_hooks
    import antenv

    if "/opt/trn_rl_repo/antenv" not in antenv.__path__:
        antenv.__path__.append("/opt/trn_rl_repo/antenv")
except Exception:
    pass

f32 = mybir.dt.float32
f32r = mybir.dt.float32r
AF = mybir.ActivationFunctionType
ALU = mybir.AluOpType

NC = 8          # cores
B = 2           # batch
T = 2048        # sequence length
C = 1024        # model dim
H = 16          # heads
HD = 64         # head dim
HPC = H // NC   # heads per core (2)
DH = HPC * HD   # per-core head cols (128)
DFF = 4096
TOK = B * T     # 4096 tokens
TOKC = TOK // NC  # 512 tokens per core
CT = C // 128   # 8 c-tiles
FT = DFF // 128  # 32 ff-tiles
KT = T // 128   # 16 k-tiles per batch
QC = T // 512   # 4 q-chunks of 512 per batch
LN_EPS = 1e-5

_CACHE = {}


def _build(debug=False):
    nc = bacc.Bacc("TRN2", target_bir_lowering=False, debug=False, num_devices=NC)

    # ---- DRAM I/O (per-core values supplied via in_maps) ----
    xt_d = nc.dram_tensor("xt", [C, TOK], f32r, kind="ExternalInput")
    wq_d = nc.dram_tensor("wq_c", [128, CT, 128], f32r, kind="ExternalInput")
    wk_d = nc.dram_tensor("wk_c", [128, CT, 128], f32r, kind="ExternalInput")
    wv_d = nc.dram_tensor("wv_c", [128, CT, 128], f32r, kind="ExternalInput")
    xres_d = nc.dram_tensor("xres_c", [C, TOKC], f32r, kind="ExternalInput")
    wp_d = nc.dram_tensor("wproj", [C, C], f32r, kind="ExternalInput")
    w1_d = nc.dram_tensor("w1p", [FT, 128, CT, 128], f32r, kind="ExternalInput")
    w2_d = nc.dram_tensor("w2", [DFF, C], f32r, kind="ExternalInput")
    bias_d = nc.dram_tensor("biaspack", [128, 6 * CT], f32, kind="ExternalInput")
    b1_d = nc.dram_tensor("b1t", [128, FT], f32, kind="ExternalInput")
    ones_d = nc.dram_tensor("onesp", [128, 128], f32r, kind="ExternalInput")
    ident_d = nc.dram_tensor("ident", [128, 128], f32, kind="ExternalInput")
    mask_d = nc.dram_tensor("cmask", [128, 4, 512], f32r, kind="ExternalInput")
    out_d = nc.dram_tensor("out", [C, TOKC], f32, kind="ExternalOutput")
    if debug:
        dbg_attn = nc.dram_tensor("dbg_attn", [2, HD, TOK], f32, kind="ExternalOutput")
        dbg_qkv = nc.dram_tensor("dbg_qkv", [3, DH, TOK], f32, kind="ExternalOutput")

    with tile.TileContext(nc) as tc:
        with (
            nc.allow_low_precision(reason="float32r matmul inputs (~6e-5 rounding)"),
            tc.tile_pool(name="const", bufs=1) as p_const,
            tc.tile_pool(name="ln1p", bufs=CT) as p_ln1,
            tc.tile_pool(name="dram", bufs=1, space="DRAM") as p_dram,
        ):
            # ---- persistent constants ----
            ones = p_const.tile([128, 128], f32r, tag="ones")
            nc.sync.dma_start(ones[:], ones_d[:])
            biasp = p_const.tile([128, 6 * CT], f32, tag="biasp")
            nc.sync.dma_start(biasp[:], bias_d[:])
            b1t = p_const.tile([128, FT], f32, tag="b1t")
            nc.sync.dma_start(b1t[:], b1_d[:])
            # bias pack columns: [bproj | b2 | g1 | be1 | g2 | be2]
            bproj_b = biasp[:, 0 * CT:1 * CT]
            b2_b = biasp[:, 1 * CT:2 * CT]
            g1_b = biasp[:, 2 * CT:3 * CT]
            be1_b = biasp[:, 3 * CT:4 * CT]
            g2_b = biasp[:, 4 * CT:5 * CT]
            be2_b = biasp[:, 5 * CT:6 * CT]

            a2a_in = p_dram.tile([NC, DH, TOKC], f32r, tag="a2ai")
            a2a_out = p_dram.tile([NC, DH, TOKC], f32r, tag="a2ao")

            # ======== phase 1: QKV + attention (head-parallel) ========
            with (
                tc.tile_pool(name="attn", bufs=1) as p_attn,
                tc.tile_pool(name="p1c", bufs=1) as p1c,
                tc.tile_pool(name="xt", bufs=8) as p_xt,
                tc.tile_pool(name="qkv", bufs=1) as p_qkv,
                tc.tile_pool(name="es", bufs=3) as p_es,
                tc.tile_pool(name="small", bufs=2) as p_small,
                tc.tile_pool(name="ps1", bufs=6, space="PSUM") as ps1,
            ):
                # per-head attention outputs (feature-major rows 0-63)
                attnh = [
                    p_attn.tile([HD, TOK], f32r, tag=f"attn{h}", name=f"attnh{h}")
                    for h in range(HPC)
                ]
                ident = p1c.tile([128, 128], f32, tag="ident")
                nc.sync.dma_start(ident[:], ident_d[:])
                masks = p1c.tile([128, 4, 512], f32r, tag="masks")
                nc.sync.dma_start(masks[:], mask_d[:])
                wq_sb = p1c.tile([128, CT, 128], f32r, tag="wq")
                wk_sb = p1c.tile([128, CT, 128], f32r, tag="wk")
                wv_sb = p1c.tile([128, CT, 128], f32r, tag="wv")
                nc.sync.dma_start(wq_sb[:], wq_d[:])
                nc.sync.dma_start(wk_sb[:], wk_d[:])
                nc.sync.dma_start(wv_sb[:], wv_d[:])

                for b in range(B):
                    qT = p_qkv.tile([DH, T], f32r, tag="q")
                    kT = p_qkv.tile([DH, T], f32r, tag="k")
                    vT = p_qkv.tile([DH, T], f32, tag="v")

                    # ---- QKV projections (feature-major), x^T streamed per
                    # (ct, chunk) so only ~2 chunks of x^T are resident ----
                    for n in range(QC):
                        ncol = slice(n * 512, (n + 1) * 512)
                        xt_sb = []
                        for ct in range(CT):
                            t = p_xt.tile([128, 512], f32r, tag="xt", bufs=16)
                            nc.sync.dma_start(
                                t[:],
                                xt_d[ct * 128:(ct + 1) * 128,
                                     b * T + n * 512:b * T + (n + 1) * 512],
                            )
                            xt_sb.append(t)
                        pq = ps1.tile([128, 512], f32, tag="sps", bufs=3)
                        pk = ps1.tile([128, 512], f32, tag="sps", bufs=3)
                        for ct in range(CT):
                            st = ct == 0
                            sp = ct == CT - 1
                            nc.tensor.matmul(
                                pq[:], wq_sb[:, ct, :], xt_sb[ct][:],
                                start=st, stop=sp,
                            )
                            nc.tensor.matmul(
                                pk[:], wk_sb[:, ct, :], xt_sb[ct][:],
                                start=st, stop=sp,
                            )
                        nc.vector.tensor_copy(qT[:, ncol], pq[:])
                        nc.vector.tensor_copy(kT[:, ncol], pk[:])
                        pv = ps1.tile([128, 512], f32, tag="sps", bufs=3)
                        for ct in range(CT):
                            nc.tensor.matmul(
                                pv[:], wv_sb[:, ct, :], xt_sb[ct][:],
                                start=(ct == 0), stop=(ct == CT - 1),
                            )
                        nc.scalar.copy(vT[:, ncol], pv[:])

                    # ---- V -> token-major; slot: [Vh0|ones|pad|Vh1|ones|pad] ----
                    SL = 132
                    vt = p_qkv.tile([128, KT, SL], f32r, tag="vt")
                    for kt in range(KT):
                        pt = ps1.tile([128, 128], f32, tag="oacc", bufs=2)
                        nc.tensor.transpose(
                            pt[:], vT[:, kt * 128:(kt + 1) * 128], ident[:]
                        )
                        nc.vector.tensor_copy(vt[:, kt, 0:64], pt[:, 0:64])
                        nc.vector.tensor_copy(vt[:, kt, 66:130], pt[:, 64:128])
                        nc.vector.tensor_copy(vt[:, kt, 64:65], ones[:, 0:1])
                        nc.vector.tensor_copy(vt[:, kt, 130:131], ones[:, 1:2])

                    # ---- causal attention: both heads interleaved per k-tile
                    # (adjacent score matmuls pack into disjoint array rows) ----
                    for j in range(QC):
                        nkt = 4 * j + 4
                        oacc = [
                            ps1.tile([65, 512], f32, tag="oacc", bufs=2,
                                     name=f"oacc{h}")
                            for h in range(HPC)
                        ]
                        for kt in range(nkt):
                            m = kt - 4 * j
                            # both heads' scores into one 2-bank PSUM tile so a
                            # single exp (and one wait) covers both attnV MMs
                            spair = ps1.tile([128, 2, 512], f32, tag="sps", bufs=3)
                            for h in range(HPC):
                                hrow = slice(h * 64, (h + 1) * 64)
                                nc.tensor.matmul(
                                    spair[:, h, :],
                                    kT[hrow, kt * 128:(kt + 1) * 128],
                                    qT[hrow, j * 512:(j + 1) * 512],
                                    start=True, stop=True,
                                    tile_position=(64 * h, 0),
                                )
                            epair = p_es.tile([128, 2, 512], f32r, tag="es", bufs=6)
                            if m < 0:
                                nc.scalar.activation(
                                    epair[:], spair[:], AF.Exp, scale=0.125
                                )
                            else:
                                ed = p_es.tile([128, 2, 512], f32, tag="esd", bufs=2)
                                nc.scalar.activation(
                                    ed[:], spair[:], AF.Exp, scale=0.125
                                )
                                for h in range(HPC):
                                    nc.vector.tensor_mul(
                                        epair[:, h, :], ed[:, h, :],
                                        masks[:, m, :].bitcast(f32),
                                    )
                            for h in range(HPC):
                                nc.tensor.matmul(
                                    oacc[h][:],
                                    vt[:, kt, 66 * h:66 * h + 65],
                                    epair[:, h, :],
                                    start=(kt == 0), stop=(kt == nkt - 1),
                                )
                        for h in range(HPC):
                            # free the PSUM accum early via two same-base copies
                            osb = p_small.tile([64, 512], f32, tag="osb", bufs=4)
                            nc.scalar.copy(osb[:], oacc[h][0:64, :])
                            sr = p_small.tile([128, 512], f32r, tag="sr")
                            nc.vector.tensor_copy(sr[64:65, :], oacc[h][64:65, :])
                            # broadcast sums across partitions, then approx-recip
                            bps = ps1.tile([64, 512], f32, tag="oacc", bufs=2)
                            nc.tensor.matmul(
                                bps[:], ones[64:65, 0:64], sr[64:65, :],
                                start=True, stop=True, tile_position=(64, 0),
                            )
                            ibc = p_small.tile([64, 512], f32, tag="ibc")
                            nc.vector.reciprocal_approx_fast(ibc[:], bps[:])
                            nc.vector.tensor_mul(
                                attnh[h][:, b * T + j * 512:b * T + (j + 1) * 512],
                                osb[:],
                                ibc[:],
                            )

                    if debug:
                        nc.sync.dma_start(
                            dbg_qkv[0, :, b * T:(b + 1) * T], qT[:].bitcast(f32)
                        )
                        nc.sync.dma_start(
                            dbg_qkv[1, :, b * T:(b + 1) * T], kT[:].bitcast(f32)
                        )
                        nc.sync.dma_start(dbg_qkv[2, :, b * T:(b + 1) * T], vT[:])

                if debug:
                    for h in range(HPC):
                        nc.sync.dma_start(dbg_attn[h], attnh[h][:].bitcast(f32))

                # ---- AllToAll staging (inside phase 1: reads attnh) ----
                for j in range(NC):
                    for h in range(HPC):
                        nc.sync.dma_start(
                            a2a_in[j, h * 64:(h + 1) * 64, :],
                            attnh[h][:, j * TOKC:(j + 1) * TOKC],
                        )
                nc.gpsimd.collective_compute(
                    "AllToAll",
                    ALU.bypass,
                    replica_groups=[list(range(NC))],
                    ins=[a2a_in[:].opt()],
                    outs=[a2a_out[:].opt()],
                )

            # ======== phase 2: gather + proj + LN1 ========
            with (
                tc.tile_pool(name="agg", bufs=8) as p_agg,
                tc.tile_pool(name="wbig", bufs=8) as p_wbig,
                tc.tile_pool(name="act2", bufs=8) as p_act2,
                tc.tile_pool(name="tmp2", bufs=2) as p_tmp2,
                tc.tile_pool(name="ps2", bufs=6, space="PSUM") as ps2,
            ):
                ag = []
                for i in range(NC):
                    t = p_agg.tile([128, TOKC], f32r, tag="ag")
                    nc.sync.dma_start(t[:], a2a_out[i])
                    ag.append(t)

                wp_sb = []
                for kt in range(CT):
                    t = p_wbig.tile([128, C], f32r, tag="wp")
                    nc.sync.dma_start(t[:], wp_d[kt * 128:(kt + 1) * 128, :])
                    wp_sb.append(t)

                xres = []
                for ct in range(CT):
                    t = p_act2.tile([128, TOKC], f32r, tag="xres")
                    nc.sync.dma_start(t[:], xres_d[ct * 128:(ct + 1) * 128, :])
                    xres.append(t)

                def layer_norm(x_tiles, g_b, be_b, out_dtype, out_pool, out_tag,
                               sq_pool, tmp_pool, ps_pool):
                    """Feature-major LN across CT tiles of [128, TOKC]."""
                    s1 = ps_pool.tile([1, TOKC], f32, tag="ps")
                    s2 = ps_pool.tile([1, TOKC], f32, tag="ps")
                    for ct in range(CT):
                        nc.tensor.matmul(
                            s1[:], ones[:, 0:1], x_tiles[ct][:],
                            start=(ct == 0), stop=(ct == CT - 1),
                        )
                    for ct in range(CT):
                        sq = sq_pool.tile([128, TOKC], f32r, tag="sq")
                        nc.vector.tensor_mul(
                            sq[:],
                            x_tiles[ct][:].bitcast(f32),
                            x_tiles[ct][:].bitcast(f32),
                        )
                        nc.tensor.matmul(
                            s2[:], ones[:, 0:1], sq[:],
                            start=(ct == 0), stop=(ct == CT - 1),
                        )
                    nmu = tmp_pool.tile([1, TOKC], f32r, tag="nmu")
                    nc.vector.tensor_scalar_mul(nmu[:], s1[:], -1.0 / C)
                    ex2 = tmp_pool.tile([1, TOKC], f32, tag="ex2")
                    nc.vector.tensor_scalar_mul(ex2[:], s2[:], 1.0 / C)
                    mu2 = tmp_pool.tile([1, TOKC], f32, tag="mu2")
                    nc.vector.tensor_mul(
                        mu2[:], nmu[:].bitcast(f32), nmu[:].bitcast(f32)
                    )
                    var = tmp_pool.tile([1, TOKC], f32, tag="var")
                    nc.vector.tensor_sub(var[:], ex2[:], mu2[:])
                    nc.vector.tensor_scalar_add(var[:], var[:], LN_EPS)
                    sd = tmp_pool.tile([1, TOKC], f32r, tag="sd")
                    nc.scalar.activation(sd[:], var[:], AF.Sqrt, bias=0.0)
                    bmu = ps_pool.tile([128, TOKC], f32, tag="ps")
                    nc.tensor.matmul(
                        bmu[:], ones[0:1, :], nmu[:], start=True, stop=True
                    )
                    brs = ps_pool.tile([128, TOKC], f32, tag="ps")
                    nc.tensor.matmul(
                        brs[:], ones[0:1, :], sd[:], start=True, stop=True
                    )
                    bmu_sb = tmp_pool.tile([128, TOKC], f32, tag="bmu")
                    nc.scalar.copy(bmu_sb[:], bmu[:])
                    brs_sb = tmp_pool.tile([128, TOKC], f32, tag="brs")
                    nc.vector.reciprocal_approx_fast(brs_sb[:], brs[:])
                    outs = []
                    for ct in range(CT):
                        t1 = tmp_pool.tile([128, TOKC], f32, tag="lntmp")
                        nc.vector.tensor_add(
                            t1[:], x_tiles[ct][:].bitcast(f32), bmu_sb[:]
                        )
                        t2 = tmp_pool.tile([128, TOKC], f32, tag="lntmp2")
                        nc.vector.tensor_mul(t2[:], t1[:], brs_sb[:])
                        o = out_pool.tile([128, TOKC], out_dtype, tag=out_tag)
                        nc.scalar.activation(
                            o[:], t2[:], AF.Identity,
                            bias=be_b[:, ct:ct + 1], scale=g_b[:, ct:ct + 1],
                        )
                        outs.append(o)
                    return outs

                # ---- proj + bias + residual ----
                x1 = []
                for mt in range(CT):
                    yps = ps2.tile([128, TOKC], f32, tag="ps")
                    for kt in range(CT):
                        nc.tensor.matmul(
                            yps[:], wp_sb[kt][:, mt * 128:(mt + 1) * 128], ag[kt][:],
                            start=(kt == 0), stop=(kt == CT - 1),
                        )
                    t1 = p_tmp2.tile([128, TOKC], f32, tag="projt")
                    nc.scalar.activation(
                        t1[:], yps[:], AF.Identity, bias=bproj_b[:, mt:mt + 1]
                    )
                    xr = p_act2.tile([128, TOKC], f32r, tag="x1")
                    nc.vector.tensor_add(xr[:], t1[:], xres[mt][:].bitcast(f32))
                    x1.append(xr)

                ln1 = layer_norm(
                    x1, g1_b, be1_b, f32r, p_ln1, "ln1", p_tmp2, p_tmp2, ps2
                )

            # ======== phase 3: FFN + LN2 + out ========
            with (
                tc.tile_pool(name="hff", bufs=FT) as p_hff,
                tc.tile_pool(name="w1s", bufs=4) as p_w1,
                tc.tile_pool(name="w2s", bufs=4) as p_w2,
                tc.tile_pool(name="act3", bufs=8) as p_act3,
                tc.tile_pool(name="tmp3", bufs=2) as p_tmp3,
                tc.tile_pool(name="outp", bufs=2) as p_out,
            ):
                # ---- FFN1: h = relu(w1^T @ ln1 + b1) ----
                hff = []
                with tc.tile_pool(name="ps3a", bufs=4, space="PSUM") as ps3a:
                    for mt in range(FT):
                        w1t = p_w1.tile([128, CT, 128], f32r, tag="w1")
                        nc.sync.dma_start(w1t[:], w1_d[mt])
                        yps = ps3a.tile([128, TOKC], f32, tag="ps")
                        for kt in range(CT):
                            nc.tensor.matmul(
                                yps[:], w1t[:, kt, :], ln1[kt][:],
                                start=(kt == 0), stop=(kt == CT - 1),
                            )
                        hf = p_hff.tile([128, TOKC], f32r, tag="hff")
                        nc.scalar.activation(
                            hf[:], yps[:], AF.Relu, bias=b1t[:, mt:mt + 1]
                        )
                        hff.append(hf)

                # ---- FFN2 (kt-outer, 8 accumulators) + LN2 ----
                with tc.tile_pool(name="ps3b", bufs=8, space="PSUM") as ps3b:
                    accs = [
                        ps3b.tile([128, TOKC], f32, tag="ps", name=f"acc{mt}")
                        for mt in range(CT)
                    ]
                    for kt in range(FT):
                        w2t = p_w2.tile([128, C], f32r, tag="w2")
                        nc.sync.dma_start(w2t[:], w2_d[kt * 128:(kt + 1) * 128, :])
                        for mt in range(CT):
                            nc.tensor.matmul(
                                accs[mt][:],
                                w2t[:, mt * 128:(mt + 1) * 128],
                                hff[kt][:],
                                start=(kt == 0), stop=(kt == FT - 1),
                            )
                    x2 = []
                    for mt in range(CT):
                        t1 = p_tmp3.tile([128, TOKC], f32, tag="ffn2t")
                        nc.scalar.activation(
                            t1[:], accs[mt][:], AF.Identity, bias=b2_b[:, mt:mt + 1]
                        )
                        xr = p_act3.tile([128, TOKC], f32r, tag="x2")
                        nc.vector.tensor_add(xr[:], t1[:], ln1[mt][:].bitcast(f32))
                        x2.append(xr)

                    out_tiles = layer_norm(
                        x2, g2_b, be2_b, f32, p_out, "outt", p_tmp3, p_tmp3, ps3b
                    )
                    for ct in range(CT):
                        nc.sync.dma_start(
                            out_d[ct * 128:(ct + 1) * 128, :], out_tiles[ct][:]
                        )

    nc.compile()
    return nc


def _pack_inputs(inputs):
    """Host-side sharding/marshalling. Returns in_maps for the 8 cores."""
    x = np.asarray(inputs["x"], dtype=np.float32)
    xf = np.ascontiguousarray(x.reshape(TOK, C))
    xt = np.ascontiguousarray(xf.T)  # [C, TOK]
    wq = np.asarray(inputs["wq"], dtype=np.float32)
    wk = np.asarray(inputs["wk"], dtype=np.float32)
    wv = np.asarray(inputs["wv"], dtype=np.float32)
    wproj = np.ascontiguousarray(np.asarray(inputs["w_proj"], dtype=np.float32))
    w1 = np.asarray(inputs["w1"], dtype=np.float32)
    w2 = np.ascontiguousarray(np.asarray(inputs["w2"], dtype=np.float32))
    # w1 packed per ff-tile: [FT, 128(p), CT, 128(f)];  w1 is [C, DFF]
    w1p = np.ascontiguousarray(
        w1.reshape(CT, 128, FT, 128).transpose(2, 1, 0, 3)
    )

    def tile_vec(v, n):
        return np.ascontiguousarray(
            np.asarray(v, dtype=np.float32).reshape(n, 128).T
        )

    biaspack = np.zeros((128, 6 * CT), dtype=np.float32)
    biaspack[:, 0 * CT:1 * CT] = tile_vec(inputs["b_proj"], CT)
    biaspack[:, 1 * CT:2 * CT] = tile_vec(inputs["b2"], CT)
    biaspack[:, 2 * CT:3 * CT] = tile_vec(inputs["g1"], CT)
    biaspack[:, 3 * CT:4 * CT] = tile_vec(inputs["be1"], CT)
    biaspack[:, 4 * CT:5 * CT] = tile_vec(inputs["g2"], CT)
    biaspack[:, 5 * CT:6 * CT] = tile_vec(inputs["be2"], CT)
    b1t = tile_vec(inputs["b1"], FT)

    # causal masks for the 4 diagonal offsets, packed [128, 4, 512]
    r = np.arange(128)[:, None]
    ccol = np.arange(512)[None, :]
    cmask = np.stack(
        [(ccol >= r + 128 * m).astype(np.float32) for m in range(4)], axis=1
    )
    cmask = np.ascontiguousarray(cmask)  # [128, 4, 512]
    onesp = np.ones((128, 128), dtype=np.float32)
    ident = np.eye(128, dtype=np.float32)

    in_maps = []
    for c in range(NC):
        hcol = slice(c * DH, (c + 1) * DH)

        def pack_w(w):
            return np.ascontiguousarray(
                w[:, hcol].reshape(CT, 128, DH).transpose(1, 0, 2)
            )

        in_maps.append(
            {
                "xt": xt,
                "wq_c": pack_w(wq),
                "wk_c": pack_w(wk),
                "wv_c": pack_w(wv),
                "xres_c": np.ascontiguousarray(
                    xt[:, c * TOKC:(c + 1) * TOKC]
                ),
                "wproj": wproj,
                "w1p": w1p,
                "w2": w2,
                "biaspack": biaspack,
                "b1t": b1t,
                "onesp": onesp,
                "ident": ident,
                "cmask": cmask,
            }
        )
    return in_maps


def _run(inputs, trace=False, debug=False):
    key = "dbg" if debug else "rel"
    if key not in _CACHE:
        _CACHE[key] = _build(debug=debug)
    nc = _CACHE[key]
    in_maps = _pack_inputs(inputs)
    res = bass_utils.run_bass_kernel_spmd(
        nc, in_maps, core_ids=list(range(NC)), trace=trace
    )
    out = np.empty((TOK, C), dtype=np.float32)
    for c in range(NC):
        out[c * TOKC:(c + 1) * TOKC, :] = res.results[c]["out"].T
    return out.reshape(B, T, C), res


def kernel(**inputs) -> np.ndarray:
    out, _ = _run(inputs, trace=False, debug=False)
    return out


# revision 24
# speedup vs baseline: 1.3569x; 1.1031x over previous
"""Trainium2 Bass kernel for a dense transformer block (B=2, T=2048, C=1024,
H=16, Dff=4096), SPMD across 8 NeuronCores.

Sharding: attention is head-parallel (2 heads/core); one AllToAll
redistributes the attention output into a token-parallel layout; projection,
layernorms and the FFN then run on each core's 512-token slice with full
weights. All on-device activations are kept feature-major (transposed) so
every matmul consumes weights exactly as stored; the host performs the
x -> x^T and out^T -> out transposes during input/output marshalling.
Matmuls run in float32r (fp32 storage, FP22 multiply, fp32 accumulate).
"""

import sys

sys.path.insert(0, "/opt/trn_rl_repo")

import numpy as np
import concourse.bacc as bacc
import concourse.mybir as mybir
import concourse.tile as tile
import concourse.bass_utils as bass_utils

try:  # make the NTFF profile shim importable as antenv.axon_hooks
    import antenv

    if "/opt/trn_rl_repo/antenv" not in antenv.__path__:
        antenv.__path__.append("/opt/trn_rl_repo/antenv")
except Exception:
    pass

f32 = mybir.dt.float32
f32r = mybir.dt.float32r
AF = mybir.ActivationFunctionType
ALU = mybir.AluOpType

NC = 8          # cores
B = 2           # batch
T = 2048        # sequence length
C = 1024        # model dim
H = 16          # heads
HD = 64         # head dim
HPC = H // NC   # heads per core (2)
DH = HPC * HD   # per-core head cols (128)
DFF = 4096
TOK = B * T     # 4096 tokens
TOKC = TOK // NC  # 512 tokens per core
CT = C // 128   # 8 c-tiles
FT = DFF // 128  # 32 ff-tiles
KT = T // 128   # 16 k-tiles per batch
QC = T // 512   # 4 q-chunks of 512 per batch
LN_EPS = 1e-5

_CACHE = {}


def _build(debug=False):
    nc = bacc.Bacc("TRN2", target_bir_lowering=False, debug=False, num_devices=NC)

    # ---- DRAM I/O (per-core values supplied via in_maps) ----
    xt_d = nc.dram_tensor("xt", [C, TOK], f32r, kind="ExternalInput")
    wq_d = nc.dram_tensor("wq_c", [128, CT, 128], f32r, kind="ExternalInput")
    wk_d = nc.dram_tensor("wk_c", [128, CT, 128], f32r, kind="ExternalInput")
    wv_d = nc.dram_tensor("wv_c", [128, CT, 128], f32r, kind="ExternalInput")
    xres_d = nc.dram_tensor("xres_c", [C, TOKC], f32r, kind="ExternalInput")
    wp_d = nc.dram_tensor("wproj", [C, C], f32r, kind="ExternalInput")
    w1_d = nc.dram_tensor("w1p", [FT, 128, CT, 128], f32r, kind="ExternalInput")
    w2_d = nc.dram_tensor("w2", [DFF, C], f32r, kind="ExternalInput")
    bias_d = nc.dram_tensor("biaspack", [128, 6 * CT], f32, kind="ExternalInput")
    b1_d = nc.dram_tensor("b1t", [128, FT], f32, kind="ExternalInput")
    ones_d = nc.dram_tensor("onesp", [128, 128], f32r, kind="ExternalInput")
    ident_d = nc.dram_tensor("ident", [128, 128], f32, kind="ExternalInput")
    mask_d = nc.dram_tensor("cmask", [128, 4, 512], f32r, kind="ExternalInput")
    out_d = nc.dram_tensor("out", [C, TOKC], f32, kind="ExternalOutput")
    if debug:
        dbg_attn = nc.dram_tensor("dbg_attn", [2, HD, TOK], f32, kind="ExternalOutput")
        dbg_qkv = nc.dram_tensor("dbg_qkv", [3, DH, TOK], f32, kind="ExternalOutput")

    with tile.TileContext(nc) as tc:
        with (
            nc.allow_low_precision(reason="float32r matmul inputs (~6e-5 rounding)"),
            tc.tile_pool(name="const", bufs=1) as p_const,
            tc.tile_pool(name="ln1p", bufs=CT) as p_ln1,
            tc.tile_pool(name="dram", bufs=1, space="DRAM") as p_dram,
        ):
            # ---- persistent constants ----
            ones = p_const.tile([128, 128], f32r, tag="ones")
            nc.sync.dma_start(ones[:], ones_d[:])
            biasp = p_const.tile([128, 6 * CT], f32, tag="biasp")
            nc.sync.dma_start(biasp[:], bias_d[:])
            b1t = p_const.tile([128, FT], f32, tag="b1t")
            nc.sync.dma_start(b1t[:], b1_d[:])
            # bias pack columns: [bproj | b2 | g1 | be1 | g2 | be2]
            bproj_b = biasp[:, 0 * CT:1 * CT]
            b2_b = biasp[:, 1 * CT:2 * CT]
            g1_b = biasp[:, 2 * CT:3 * CT]
            be1_b = biasp[:, 3 * CT:4 * CT]
            g2_b = biasp[:, 4 * CT:5 * CT]
            be2_b = biasp[:, 5 * CT:6 * CT]

            # two half-AllToAlls: batch-0 shards exchange while batch-1
            # attention still computes. Core c owns tokens
            # [c*256,(c+1)*256) of each batch (512 total).
            HT = TOKC // B  # 256 tokens per batch per core
            a2a_in = [
                p_dram.tile([NC, DH, HT], f32r, tag=f"a2ai{b}", name=f"a2ai{b}")
                for b in range(B)
            ]
            a2a_out = [
                p_dram.tile([NC, DH, HT], f32r, tag=f"a2ao{b}", name=f"a2ao{b}")
                for b in range(B)
            ]

            # ======== phase 1: QKV + attention (head-parallel) ========
            with (
                tc.tile_pool(name="attn", bufs=1) as p_attn,
                tc.tile_pool(name="p1c", bufs=1) as p1c,
                tc.tile_pool(name="xt", bufs=8) as p_xt,
                tc.tile_pool(name="qkv", bufs=1) as p_qkv,
                tc.tile_pool(name="es", bufs=3) as p_es,
                tc.tile_pool(name="small", bufs=2) as p_small,
                tc.tile_pool(name="ps1", bufs=6, space="PSUM") as ps1,
            ):
                # per-head attention outputs (feature-major rows 0-63)
                attnh = [
                    p_attn.tile([HD, TOK], f32r, tag=f"attn{h}", name=f"attnh{h}")
                    for h in range(HPC)
                ]
                ident = p1c.tile([128, 128], f32, tag="ident")
                nc.sync.dma_start(ident[:], ident_d[:])
                masks = p1c.tile([128, 4, 512], f32r, tag="masks")
                nc.sync.dma_start(masks[:], mask_d[:])
                wq_sb = p1c.tile([128, CT, 128], f32r, tag="wq")
                wk_sb = p1c.tile([128, CT, 128], f32r, tag="wk")
                wv_sb = p1c.tile([128, CT, 128], f32r, tag="wv")
                nc.sync.dma_start(wq_sb[:], wq_d[:])
                nc.sync.dma_start(wk_sb[:], wk_d[:])
                nc.sync.dma_start(wv_sb[:], wv_d[:])

                for b in range(B):
                    qT = p_qkv.tile([DH, T], f32r, tag="q")
                    kT = p_qkv.tile([DH, T], f32r, tag="k")
                    vT = p_qkv.tile([DH, T], f32, tag="v")

                    # ---- QKV projections (feature-major), x^T streamed per
                    # (ct, chunk) so only ~2 chunks of x^T are resident ----
                    for n in range(QC):
                        ncol = slice(n * 512, (n + 1) * 512)
                        xt_sb = []
                        for ct in range(CT):
                            t = p_xt.tile([128, 512], f32r, tag="xt", bufs=16)
                            nc.sync.dma_start(
                                t[:],
                                xt_d[ct * 128:(ct + 1) * 128,
                                     b * T + n * 512:b * T + (n + 1) * 512],
                            )
                            xt_sb.append(t)
                        pq = ps1.tile([128, 512], f32, tag="sps", bufs=3)
                        pk = ps1.tile([128, 512], f32, tag="sps", bufs=3)
                        for ct in range(CT):
                            st = ct == 0
                            sp = ct == CT - 1
                            nc.tensor.matmul(
                                pq[:], wq_sb[:, ct, :], xt_sb[ct][:],
                                start=st, stop=sp,
                            )
                            nc.tensor.matmul(
                                pk[:], wk_sb[:, ct, :], xt_sb[ct][:],
                                start=st, stop=sp,
                            )
                        nc.vector.tensor_copy(qT[:, ncol], pq[:])
                        nc.vector.tensor_copy(kT[:, ncol], pk[:])
                        pv = ps1.tile([128, 512], f32, tag="sps", bufs=3)
                        for ct in range(CT):
                            nc.tensor.matmul(
                                pv[:], wv_sb[:, ct, :], xt_sb[ct][:],
                                start=(ct == 0), stop=(ct == CT - 1),
                            )
                        nc.scalar.copy(vT[:, ncol], pv[:])

                    # ---- V -> token-major; slot: [Vh0|ones|pad|Vh1|ones|pad] ----
                    SL = 132
                    vt = p_qkv.tile([128, KT, SL], f32r, tag="vt")
                    for kt in range(KT):
                        pt = ps1.tile([128, 128], f32, tag="oacc", bufs=2)
                        nc.tensor.transpose(
                            pt[:], vT[:, kt * 128:(kt + 1) * 128], ident[:]
                        )
                        nc.vector.tensor_copy(vt[:, kt, 0:64], pt[:, 0:64])
                        nc.vector.tensor_copy(vt[:, kt, 66:130], pt[:, 64:128])
                        nc.vector.tensor_copy(vt[:, kt, 64:65], ones[:, 0:1])
                        nc.vector.tensor_copy(vt[:, kt, 130:131], ones[:, 1:2])

                    # ---- causal attention: both heads interleaved per k-tile
                    # (adjacent score matmuls pack into disjoint array rows) ----
                    for j in range(QC):
                        nkt = 4 * j + 4
                        oacc = [
                            ps1.tile([65, 512], f32, tag="oacc", bufs=2,
                                     name=f"oacc{h}")
                            for h in range(HPC)
                        ]
                        for kt in range(nkt):
                            m = kt - 4 * j
                            # both heads' scores into one 2-bank PSUM tile so a
                            # single exp (and one wait) covers both attnV MMs
                            spair = ps1.tile([128, 2, 512], f32, tag="sps", bufs=3)
                            for h in range(HPC):
                                hrow = slice(h * 64, (h + 1) * 64)
                                nc.tensor.matmul(
                                    spair[:, h, :],
                                    kT[hrow, kt * 128:(kt + 1) * 128],
                                    qT[hrow, j * 512:(j + 1) * 512],
                                    start=True, stop=True,
                                    tile_position=(64 * h, 0),
                                )
                            epair = p_es.tile([128, 2, 512], f32r, tag="es", bufs=6)
                            if m < 0:
                                nc.scalar.activation(
                                    epair[:], spair[:], AF.Exp, scale=0.125
                                )
                            else:
                                ed = p_es.tile([128, 2, 512], f32, tag="esd", bufs=2)
                                nc.scalar.activation(
                                    ed[:], spair[:], AF.Exp, scale=0.125
                                )
                                for h in range(HPC):
                                    nc.vector.tensor_mul(
                                        epair[:, h, :], ed[:, h, :],
                                        masks[:, m, :].bitcast(f32),
                                    )
                            for h in range(HPC):
                                nc.tensor.matmul(
                                    oacc[h][:],
                                    vt[:, kt, 66 * h:66 * h + 65],
                                    epair[:, h, :],
                                    start=(kt == 0), stop=(kt == nkt - 1),
                                )
                        for h in range(HPC):
                            # free the PSUM accum early via two same-base copies
                            osb = p_small.tile([64, 512], f32, tag="osb", bufs=4)
                            nc.scalar.copy(osb[:], oacc[h][0:64, :])
                            sr = p_small.tile([128, 512], f32r, tag="sr")
                            nc.vector.tensor_copy(sr[64:65, :], oacc[h][64:65, :])
                            # broadcast sums across partitions, then approx-recip
                            bps = ps1.tile([64, 512], f32, tag="oacc", bufs=2)
                            nc.tensor.matmul(
                                bps[:], ones[64:65, 0:64], sr[64:65, :],
                                start=True, stop=True, tile_position=(64, 0),
                            )
                            ibc = p_small.tile([64, 512], f32, tag="ibc")
                            nc.vector.reciprocal_approx_fast(ibc[:], bps[:])
                            nc.vector.tensor_mul(
                                attnh[h][:, b * T + j * 512:b * T + (j + 1) * 512],
                                osb[:],
                                ibc[:],
                            )

                    if debug:
                        nc.sync.dma_start(
                            dbg_qkv[0, :, b * T:(b + 1) * T], qT[:].bitcast(f32)
                        )
                        nc.sync.dma_start(
                            dbg_qkv[1, :, b * T:(b + 1) * T], kT[:].bitcast(f32)
                        )
                        nc.sync.dma_start(dbg_qkv[2, :, b * T:(b + 1) * T], vT[:])

                    # ---- AllToAll for this batch's token shards; the b=0
                    # exchange overlaps batch-1 QKV + attention ----
                    for j in range(NC):
                        for h in range(HPC):
                            nc.sync.dma_start(
                                a2a_in[b][j, h * 64:(h + 1) * 64, :],
                                attnh[h][:, b * T + j * HT:b * T + (j + 1) * HT],
                            )
                    nc.gpsimd.collective_compute(
                        "AllToAll",
                        ALU.bypass,
                        replica_groups=[list(range(NC))],
                        ins=[a2a_in[b][:].opt()],
                        outs=[a2a_out[b][:].opt()],
                    )

                if debug:
                    for h in range(HPC):
                        nc.sync.dma_start(dbg_attn[h], attnh[h][:].bitcast(f32))

            # ======== phase 2: gather + proj + LN1 ========
            with (
                tc.tile_pool(name="agg", bufs=8) as p_agg,
                tc.tile_pool(name="wbig", bufs=8) as p_wbig,
                tc.tile_pool(name="act2", bufs=8) as p_act2,
                tc.tile_pool(name="tmp2", bufs=2) as p_tmp2,
                tc.tile_pool(name="ps2", bufs=6, space="PSUM") as ps2,
            ):
                ag = []
                for i in range(NC):
                    t = p_agg.tile([128, TOKC], f32r, tag="ag")
                    nc.sync.dma_start(t[:, 0:HT], a2a_out[0][i])
                    nc.sync.dma_start(t[:, HT:TOKC], a2a_out[1][i])
                    ag.append(t)

                wp_sb = []
                for kt in range(CT):
                    t = p_wbig.tile([128, C], f32r, tag="wp")
                    nc.sync.dma_start(t[:], wp_d[kt * 128:(kt + 1) * 128, :])
                    wp_sb.append(t)

                xres = []
                for ct in range(CT):
                    t = p_act2.tile([128, TOKC], f32r, tag="xres")
                    nc.sync.dma_start(t[:], xres_d[ct * 128:(ct + 1) * 128, :])
                    xres.append(t)

                def layer_norm(x_tiles, g_b, be_b, out_dtype, out_pool, out_tag,
                               sq_pool, tmp_pool, ps_pool):
                    """Feature-major LN across CT tiles of [128, TOKC]."""
                    s1 = ps_pool.tile([1, TOKC], f32, tag="ps")
                    s2 = ps_pool.tile([1, TOKC], f32, tag="ps")
                    for ct in range(CT):
                        nc.tensor.matmul(
                            s1[:], ones[:, 0:1], x_tiles[ct][:],
                            start=(ct == 0), stop=(ct == CT - 1),
                        )
                    for ct in range(CT):
                        sq = sq_pool.tile([128, TOKC], f32r, tag="sq")
                        nc.vector.tensor_mul(
                            sq[:],
                            x_tiles[ct][:].bitcast(f32),
                            x_tiles[ct][:].bitcast(f32),
                        )
                        nc.tensor.matmul(
                            s2[:], ones[:, 0:1], sq[:],
                            start=(ct == 0), stop=(ct == CT - 1),
                        )
                    nmu = tmp_pool.tile([1, TOKC], f32r, tag="nmu")
                    nc.vector.tensor_scalar_mul(nmu[:], s1[:], -1.0 / C)
                    ex2 = tmp_pool.tile([1, TOKC], f32, tag="ex2")
                    nc.vector.tensor_scalar_mul(ex2[:], s2[:], 1.0 / C)
                    mu2 = tmp_pool.tile([1, TOKC], f32, tag="mu2")
                    nc.vector.tensor_mul(
                        mu2[:], nmu[:].bitcast(f32), nmu[:].bitcast(f32)
                    )
                    var = tmp_pool.tile([1, TOKC], f32, tag="var")
                    nc.vector.tensor_sub(var[:], ex2[:], mu2[:])
                    nc.vector.tensor_scalar_add(var[:], var[:], LN_EPS)
                    sd = tmp_pool.tile([1, TOKC], f32r, tag="sd")
                    nc.scalar.activation(sd[:], var[:], AF.Sqrt, bias=0.0)
                    bmu = ps_pool.tile([128, TOKC], f32, tag="ps")
                    nc.tensor.matmul(
                        bmu[:], ones[0:1, :], nmu[:], start=True, stop=True
                    )
                    brs = ps_pool.tile([128, TOKC], f32, tag="ps")
                    nc.tensor.matmul(
                        brs[:], ones[0:1, :], sd[:], start=True, stop=True
                    )
                    bmu_sb = tmp_pool.tile([128, TOKC], f32, tag="bmu")
                    nc.scalar.copy(bmu_sb[:], bmu[:])
                    brs_sb = tmp_pool.tile([128, TOKC], f32, tag="brs")
                    nc.vector.reciprocal_approx_fast(brs_sb[:], brs[:])
                    outs = []
                    for ct in range(CT):
                        t1 = tmp_pool.tile([128, TOKC], f32, tag="lntmp")
                        nc.vector.tensor_add(
                            t1[:], x_tiles[ct][:].bitcast(f32), bmu_sb[:]
                        )
                        t2 = tmp_pool.tile([128, TOKC], f32, tag="lntmp2")
                        nc.vector.tensor_mul(t2[:], t1[:], brs_sb[:])
                        o = out_pool.tile([128, TOKC], out_dtype, tag=out_tag)
                        nc.scalar.activation(
                            o[:], t2[:], AF.Identity,
                            bias=be_b[:, ct:ct + 1], scale=g_b[:, ct:ct + 1],
                        )
                        outs.append(o)
                    return outs

                # ---- proj + bias + residual ----
                x1 = []
                for mt in range(CT):
                    yps = ps2.tile([128, TOKC], f32, tag="ps")
                    for kt in range(CT):
                        nc.tensor.matmul(
                            yps[:], wp_sb[kt][:, mt * 128:(mt + 1) * 128], ag[kt][:],
                            start=(kt == 0), stop=(kt == CT - 1),
                        )
                    t1 = p_tmp2.tile([128, TOKC], f32, tag="projt")
                    nc.scalar.activation(
                        t1[:], yps[:], AF.Identity, bias=bproj_b[:, mt:mt + 1]
                    )
                    xr = p_act2.tile([128, TOKC], f32r, tag="x1")
                    nc.vector.tensor_add(xr[:], t1[:], xres[mt][:].bitcast(f32))
                    x1.append(xr)

                ln1 = layer_norm(
                    x1, g1_b, be1_b, f32r, p_ln1, "ln1", p_tmp2, p_tmp2, ps2
                )

            # ======== phase 3: FFN + LN2 + out ========
            with (
                tc.tile_pool(name="hff", bufs=FT) as p_hff,
                tc.tile_pool(name="w1s", bufs=4) as p_w1,
                tc.tile_pool(name="w2s", bufs=4) as p_w2,
                tc.tile_pool(name="act3", bufs=8) as p_act3,
                tc.tile_pool(name="tmp3", bufs=2) as p_tmp3,
                tc.tile_pool(name="outp", bufs=2) as p_out,
            ):
                # ---- FFN1: h = relu(w1^T @ ln1 + b1) ----
                hff = []
                with tc.tile_pool(name="ps3a", bufs=4, space="PSUM") as ps3a:
                    for mt in range(FT):
                        w1t = p_w1.tile([128, CT, 128], f32r, tag="w1")
                        nc.sync.dma_start(w1t[:], w1_d[mt])
                        yps = ps3a.tile([128, TOKC], f32, tag="ps")
                        for kt in range(CT):
                            nc.tensor.matmul(
                                yps[:], w1t[:, kt, :], ln1[kt][:],
                                start=(kt == 0), stop=(kt == CT - 1),
                            )
                        hf = p_hff.tile([128, TOKC], f32r, tag="hff")
                        nc.scalar.activation(
                            hf[:], yps[:], AF.Relu, bias=b1t[:, mt:mt + 1]
                        )
                        hff.append(hf)

                # ---- FFN2 (kt-outer, 8 accumulators) + LN2 ----
                with tc.tile_pool(name="ps3b", bufs=8, space="PSUM") as ps3b:
                    accs = [
                        ps3b.tile([128, TOKC], f32, tag="ps", name=f"acc{mt}")
                        for mt in range(CT)
                    ]
                    for kt in range(FT):
                        w2t = p_w2.tile([128, C], f32r, tag="w2")
                        nc.sync.dma_start(w2t[:], w2_d[kt * 128:(kt + 1) * 128, :])
                        for mt in range(CT):
                            nc.tensor.matmul(
                                accs[mt][:],
                                w2t[:, mt * 128:(mt + 1) * 128],
                                hff[kt][:],
                                start=(kt == 0), stop=(kt == FT - 1),
                            )
                    x2 = []
                    for mt in range(CT):
                        t1 = p_tmp3.tile([128, TOKC], f32, tag="ffn2t")
                        nc.scalar.activation(
                            t1[:], accs[mt][:], AF.Identity, bias=b2_b[:, mt:mt + 1]
                        )
                        xr = p_act3.tile([128, TOKC], f32r, tag="x2")
                        nc.vector.tensor_add(xr[:], t1[:], ln1[mt][:].bitcast(f32))
                        x2.append(xr)

                    out_tiles = layer_norm(
                        x2, g2_b, be2_b, f32, p_out, "outt", p_tmp3, p_tmp3, ps3b
                    )
                    for ct in range(CT):
                        nc.sync.dma_start(
                            out_d[ct * 128:(ct + 1) * 128, :], out_tiles[ct][:]
                        )

    nc.compile()
    return nc


def _pack_inputs(inputs):
    """Host-side sharding/marshalling. Returns in_maps for the 8 cores."""
    x = np.asarray(inputs["x"], dtype=np.float32)
    xf = np.ascontiguousarray(x.reshape(TOK, C))
    xt = np.ascontiguousarray(xf.T)  # [C, TOK]
    wq = np.asarray(inputs["wq"], dtype=np.float32)
    wk = np.asarray(inputs["wk"], dtype=np.float32)
    wv = np.asarray(inputs["wv"], dtype=np.float32)
    wproj = np.ascontiguousarray(np.asarray(inputs["w_proj"], dtype=np.float32))
    w1 = np.asarray(inputs["w1"], dtype=np.float32)
    w2 = np.ascontiguousarray(np.asarray(inputs["w2"], dtype=np.float32))
    # w1 packed per ff-tile: [FT, 128(p), CT, 128(f)];  w1 is [C, DFF]
    w1p = np.ascontiguousarray(
        w1.reshape(CT, 128, FT, 128).transpose(2, 1, 0, 3)
    )

    def tile_vec(v, n):
        return np.ascontiguousarray(
            np.asarray(v, dtype=np.float32).reshape(n, 128).T
        )

    biaspack = np.zeros((128, 6 * CT), dtype=np.float32)
    biaspack[:, 0 * CT:1 * CT] = tile_vec(inputs["b_proj"], CT)
    biaspack[:, 1 * CT:2 * CT] = tile_vec(inputs["b2"], CT)
    biaspack[:, 2 * CT:3 * CT] = tile_vec(inputs["g1"], CT)
    biaspack[:, 3 * CT:4 * CT] = tile_vec(inputs["be1"], CT)
    biaspack[:, 4 * CT:5 * CT] = tile_vec(inputs["g2"], CT)
    biaspack[:, 5 * CT:6 * CT] = tile_vec(inputs["be2"], CT)
    b1t = tile_vec(inputs["b1"], FT)

    # causal masks for the 4 diagonal offsets, packed [128, 4, 512]
    r = np.arange(128)[:, None]
    ccol = np.arange(512)[None, :]
    cmask = np.stack(
        [(ccol >= r + 128 * m).astype(np.float32) for m in range(4)], axis=1
    )
    cmask = np.ascontiguousarray(cmask)  # [128, 4, 512]
    onesp = np.ones((128, 128), dtype=np.float32)
    ident = np.eye(128, dtype=np.float32)

    in_maps = []
    for c in range(NC):
        hcol = slice(c * DH, (c + 1) * DH)

        def pack_w(w):
            return np.ascontiguousarray(
                w[:, hcol].reshape(CT, 128, DH).transpose(1, 0, 2)
            )

        in_maps.append(
            {
                "xt": xt,
                "wq_c": pack_w(wq),
                "wk_c": pack_w(wk),
                "wv_c": pack_w(wv),
                "xres_c": np.ascontiguousarray(
                    np.concatenate(
                        [
                            xt[:, b * T + c * (TOKC // B):
                               b * T + (c + 1) * (TOKC // B)]
                            for b in range(B)
                        ],
                        axis=1,
                    )
                ),
                "wproj": wproj,
                "w1p": w1p,
                "w2": w2,
                "biaspack": biaspack,
                "b1t": b1t,
                "onesp": onesp,
                "ident": ident,
                "cmask": cmask,
            }
        )
    return in_maps


def _run(inputs, trace=False, debug=False):
    key = "dbg" if debug else "rel"
    if key not in _CACHE:
        _CACHE[key] = _build(debug=debug)
    nc = _CACHE[key]
    in_maps = _pack_inputs(inputs)
    res = bass_utils.run_bass_kernel_spmd(
        nc, in_maps, core_ids=list(range(NC)), trace=trace
    )
    out = np.empty((TOK, C), dtype=np.float32)
    ht = TOKC // B
    for c in range(NC):
        oc = res.results[c]["out"]
        for b in range(B):
            out[b * T + c * ht:b * T + (c + 1) * ht, :] = (
                oc[:, b * ht:(b + 1) * ht].T
            )
    return out.reshape(B, T, C), res


def kernel(**inputs) -> np.ndarray:
    out, _ = _run(inputs, trace=False, debug=False)
    return out


# revision 26
# speedup vs baseline: 1.3934x; 1.0269x over previous
"""Trainium2 Bass kernel for a dense transformer block (B=2, T=2048, C=1024,
H=16, Dff=4096), SPMD across 8 NeuronCores.

Sharding: attention is head-parallel (2 heads/core); one AllToAll
redistributes the attention output into a token-parallel layout; projection,
layernorms and the FFN then run on each core's 512-token slice with full
weights. All on-device activations are kept feature-major (transposed) so
every matmul consumes weights exactly as stored; the host performs the
x -> x^T and out^T -> out transposes during input/output marshalling.
Matmuls run in float32r (fp32 storage, FP22 multiply, fp32 accumulate).
"""

import sys

sys.path.insert(0, "/opt/trn_rl_repo")

import numpy as np
import ml_dtypes
import concourse.bacc as bacc
import concourse.mybir as mybir
import concourse.tile as tile
import concourse.bass_utils as bass_utils

try:  # make the NTFF profile shim importable as antenv.axon_hooks
    import antenv

    if "/opt/trn_rl_repo/antenv" not in antenv.__path__:
        antenv.__path__.append("/opt/trn_rl_repo/antenv")
except Exception:
    pass

f32 = mybir.dt.float32
f32r = mybir.dt.float32r
bf16 = mybir.dt.bfloat16
AF = mybir.ActivationFunctionType
ALU = mybir.AluOpType

NC = 8          # cores
B = 2           # batch
T = 2048        # sequence length
C = 1024        # model dim
H = 16          # heads
HD = 64         # head dim
HPC = H // NC   # heads per core (2)
DH = HPC * HD   # per-core head cols (128)
DFF = 4096
TOK = B * T     # 4096 tokens
TOKC = TOK // NC  # 512 tokens per core
CT = C // 128   # 8 c-tiles
FT = DFF // 128  # 32 ff-tiles
KT = T // 128   # 16 k-tiles per batch
QC = T // 512   # 4 q-chunks of 512 per batch
LN_EPS = 1e-5

_CACHE = {}


def _build(debug=False):
    nc = bacc.Bacc("TRN2", target_bir_lowering=False, debug=False, num_devices=NC)

    # ---- DRAM I/O (per-core values supplied via in_maps) ----
    xt_d = nc.dram_tensor("xt", [C, TOK], f32r, kind="ExternalInput")
    wq_d = nc.dram_tensor("wq_c", [128, CT, 128], f32r, kind="ExternalInput")
    wk_d = nc.dram_tensor("wk_c", [128, CT, 128], f32r, kind="ExternalInput")
    wv_d = nc.dram_tensor("wv_c", [128, CT, 128], f32r, kind="ExternalInput")
    xres_d = nc.dram_tensor("xres_c", [C, TOKC], f32r, kind="ExternalInput")
    wp_d = nc.dram_tensor("wproj", [C, C], bf16, kind="ExternalInput")
    w1_d = nc.dram_tensor("w1p", [FT, 128, CT, 128], bf16, kind="ExternalInput")
    w2_d = nc.dram_tensor("w2", [DFF, C], bf16, kind="ExternalInput")
    bias_d = nc.dram_tensor("biaspack", [128, 6 * CT], f32, kind="ExternalInput")
    b1_d = nc.dram_tensor("b1t", [128, FT], f32, kind="ExternalInput")
    ones_d = nc.dram_tensor("onesp", [128, 128], f32r, kind="ExternalInput")
    ident_d = nc.dram_tensor("ident", [128, 128], f32, kind="ExternalInput")
    mask_d = nc.dram_tensor("cmask", [128, 4, 512], f32r, kind="ExternalInput")
    out_d = nc.dram_tensor("out", [C, TOKC], f32, kind="ExternalOutput")
    if debug:
        dbg_attn = nc.dram_tensor("dbg_attn", [2, HD, TOK], f32, kind="ExternalOutput")
        dbg_qkv = nc.dram_tensor("dbg_qkv", [3, DH, TOK], f32, kind="ExternalOutput")

    with tile.TileContext(nc) as tc:
        with (
            nc.allow_low_precision(reason="float32r matmul inputs (~6e-5 rounding)"),
            tc.tile_pool(name="const", bufs=1) as p_const,
            tc.tile_pool(name="ln1p", bufs=CT) as p_ln1,
            tc.tile_pool(name="dram", bufs=1, space="DRAM") as p_dram,
        ):
            # ---- persistent constants ----
            ones = p_const.tile([128, 128], f32r, tag="ones")
            nc.sync.dma_start(ones[:], ones_d[:])
            biasp = p_const.tile([128, 6 * CT], f32, tag="biasp")
            nc.sync.dma_start(biasp[:], bias_d[:])
            b1t = p_const.tile([128, FT], f32, tag="b1t")
            nc.sync.dma_start(b1t[:], b1_d[:])
            # bias pack columns: [bproj | b2 | g1 | be1 | g2 | be2]
            bproj_b = biasp[:, 0 * CT:1 * CT]
            b2_b = biasp[:, 1 * CT:2 * CT]
            g1_b = biasp[:, 2 * CT:3 * CT]
            be1_b = biasp[:, 3 * CT:4 * CT]
            g2_b = biasp[:, 4 * CT:5 * CT]
            be2_b = biasp[:, 5 * CT:6 * CT]

            # two half-AllToAlls: batch-0 shards exchange while batch-1
            # attention still computes. Core c owns tokens
            # [c*256,(c+1)*256) of each batch (512 total).
            HT = TOKC // B  # 256 tokens per batch per core
            a2a_in = [
                p_dram.tile([NC, DH, HT], bf16, tag=f"a2ai{b}", name=f"a2ai{b}")
                for b in range(B)
            ]
            a2a_out = [
                p_dram.tile([NC, DH, HT], bf16, tag=f"a2ao{b}", name=f"a2ao{b}")
                for b in range(B)
            ]

            # ======== phase 1: QKV + attention (head-parallel) ========
            with (
                tc.tile_pool(name="attn", bufs=1) as p_attn,
                tc.tile_pool(name="p1c", bufs=1) as p1c,
                tc.tile_pool(name="xt", bufs=8) as p_xt,
                tc.tile_pool(name="qkv", bufs=1) as p_qkv,
                tc.tile_pool(name="es", bufs=3) as p_es,
                tc.tile_pool(name="small", bufs=2) as p_small,
                tc.tile_pool(name="ps1", bufs=6, space="PSUM") as ps1,
            ):
                # per-head attention outputs (feature-major rows 0-63)
                attnh = [
                    p_attn.tile([HD, TOK], bf16, tag=f"attn{h}", name=f"attnh{h}")
                    for h in range(HPC)
                ]
                ident = p1c.tile([128, 128], f32, tag="ident")
                nc.sync.dma_start(ident[:], ident_d[:])
                masks = p1c.tile([128, 4, 512], f32r, tag="masks")
                nc.sync.dma_start(masks[:], mask_d[:])
                wq_sb = p1c.tile([128, CT, 128], f32r, tag="wq")
                wk_sb = p1c.tile([128, CT, 128], f32r, tag="wk")
                wv_sb = p1c.tile([128, CT, 128], f32r, tag="wv")
                nc.sync.dma_start(wq_sb[:], wq_d[:])
                nc.sync.dma_start(wk_sb[:], wk_d[:])
                nc.sync.dma_start(wv_sb[:], wv_d[:])

                for b in range(B):
                    qT = p_qkv.tile([DH, T], f32r, tag="q")
                    kT = p_qkv.tile([DH, T], f32r, tag="k")
                    vT = p_qkv.tile([DH, T], f32, tag="v")

                    # ---- QKV projections (feature-major), x^T streamed per
                    # (ct, chunk) so only ~2 chunks of x^T are resident ----
                    for n in range(QC):
                        ncol = slice(n * 512, (n + 1) * 512)
                        xt_sb = []
                        for ct in range(CT):
                            t = p_xt.tile([128, 512], f32r, tag="xt", bufs=16)
                            nc.sync.dma_start(
                                t[:],
                                xt_d[ct * 128:(ct + 1) * 128,
                                     b * T + n * 512:b * T + (n + 1) * 512],
                            )
                            xt_sb.append(t)
                        pq = ps1.tile([128, 512], f32, tag="sps", bufs=3)
                        pk = ps1.tile([128, 512], f32, tag="sps", bufs=3)
                        for ct in range(CT):
                            st = ct == 0
                            sp = ct == CT - 1
                            nc.tensor.matmul(
                                pq[:], wq_sb[:, ct, :], xt_sb[ct][:],
                                start=st, stop=sp,
                            )
                            nc.tensor.matmul(
                                pk[:], wk_sb[:, ct, :], xt_sb[ct][:],
                                start=st, stop=sp,
                            )
                        nc.vector.tensor_copy(qT[:, ncol], pq[:])
                        nc.vector.tensor_copy(kT[:, ncol], pk[:])
                        pv = ps1.tile([128, 512], f32, tag="sps", bufs=3)
                        for ct in range(CT):
                            nc.tensor.matmul(
                                pv[:], wv_sb[:, ct, :], xt_sb[ct][:],
                                start=(ct == 0), stop=(ct == CT - 1),
                            )
                        nc.scalar.copy(vT[:, ncol], pv[:])

                    # ---- V -> token-major; slot: [Vh0|ones|pad|Vh1|ones|pad] ----
                    SL = 132
                    vt = p_qkv.tile([128, KT, SL], f32r, tag="vt")
                    for kt in range(KT):
                        pt = ps1.tile([128, 128], f32, tag="oacc", bufs=2)
                        nc.tensor.transpose(
                            pt[:], vT[:, kt * 128:(kt + 1) * 128], ident[:]
                        )
                        nc.vector.tensor_copy(vt[:, kt, 0:64], pt[:, 0:64])
                        nc.vector.tensor_copy(vt[:, kt, 66:130], pt[:, 64:128])
                        nc.vector.tensor_copy(vt[:, kt, 64:65], ones[:, 0:1])
                        nc.vector.tensor_copy(vt[:, kt, 130:131], ones[:, 1:2])

                    # ---- causal attention: both heads interleaved per k-tile
                    # (adjacent score matmuls pack into disjoint array rows) ----
                    for j in range(QC):
                        nkt = 4 * j + 4
                        oacc = [
                            ps1.tile([65, 512], f32, tag="oacc", bufs=2,
                                     name=f"oacc{h}")
                            for h in range(HPC)
                        ]
                        for kt in range(nkt):
                            m = kt - 4 * j
                            # both heads' scores into one 2-bank PSUM tile so a
                            # single exp (and one wait) covers both attnV MMs
                            spair = ps1.tile([128, 2, 512], f32, tag="sps", bufs=3)
                            for h in range(HPC):
                                hrow = slice(h * 64, (h + 1) * 64)
                                nc.tensor.matmul(
                                    spair[:, h, :],
                                    kT[hrow, kt * 128:(kt + 1) * 128],
                                    qT[hrow, j * 512:(j + 1) * 512],
                                    start=True, stop=True,
                                    tile_position=(64 * h, 0),
                                )
                            epair = p_es.tile([128, 2, 512], f32r, tag="es", bufs=6)
                            if m < 0:
                                nc.scalar.activation(
                                    epair[:], spair[:], AF.Exp, scale=0.125
                                )
                            else:
                                ed = p_es.tile([128, 2, 512], f32, tag="esd", bufs=2)
                                nc.scalar.activation(
                                    ed[:], spair[:], AF.Exp, scale=0.125
                                )
                                for h in range(HPC):
                                    nc.vector.tensor_mul(
                                        epair[:, h, :], ed[:, h, :],
                                        masks[:, m, :].bitcast(f32),
                                    )
                            for h in range(HPC):
                                nc.tensor.matmul(
                                    oacc[h][:],
                                    vt[:, kt, 66 * h:66 * h + 65],
                                    epair[:, h, :],
                                    start=(kt == 0), stop=(kt == nkt - 1),
                                )
                        for h in range(HPC):
                            # free the PSUM accum early via two same-base copies
                            osb = p_small.tile([64, 512], f32, tag="osb", bufs=4)
                            nc.scalar.copy(osb[:], oacc[h][0:64, :])
                            sr = p_small.tile([128, 512], f32r, tag="sr")
                            nc.vector.tensor_copy(sr[64:65, :], oacc[h][64:65, :])
                            # broadcast sums across partitions, then approx-recip
                            bps = ps1.tile([64, 512], f32, tag="oacc", bufs=2)
                            nc.tensor.matmul(
                                bps[:], ones[64:65, 0:64], sr[64:65, :],
                                start=True, stop=True, tile_position=(64, 0),
                            )
                            ibc = p_small.tile([64, 512], f32, tag="ibc")
                            nc.vector.reciprocal_approx_fast(ibc[:], bps[:])
                            nc.vector.tensor_mul(
                                attnh[h][:, b * T + j * 512:b * T + (j + 1) * 512],
                                osb[:],
                                ibc[:],
                            )

                    if debug:
                        nc.sync.dma_start(
                            dbg_qkv[0, :, b * T:(b + 1) * T], qT[:].bitcast(f32)
                        )
                        nc.sync.dma_start(
                            dbg_qkv[1, :, b * T:(b + 1) * T], kT[:].bitcast(f32)
                        )
                        nc.sync.dma_start(dbg_qkv[2, :, b * T:(b + 1) * T], vT[:])

                    # ---- AllToAll for this batch's token shards; the b=0
                    # exchange overlaps batch-1 QKV + attention ----
                    for j in range(NC):
                        for h in range(HPC):
                            nc.sync.dma_start(
                                a2a_in[b][j, h * 64:(h + 1) * 64, :],
                                attnh[h][:, b * T + j * HT:b * T + (j + 1) * HT],
                            )
                    nc.gpsimd.collective_compute(
                        "AllToAll",
                        ALU.bypass,
                        replica_groups=[list(range(NC))],
                        ins=[a2a_in[b][:].opt()],
                        outs=[a2a_out[b][:].opt()],
                    )

                if debug:
                    for h in range(HPC):
                        nc.sync.dma_start(dbg_attn[h], attnh[h][:].bitcast(f32))

            # ======== phase 2: gather + proj + LN1 ========
            with (
                tc.tile_pool(name="agg", bufs=8) as p_agg,
                tc.tile_pool(name="wbig", bufs=8) as p_wbig,
                tc.tile_pool(name="act2", bufs=8) as p_act2,
                tc.tile_pool(name="tmp2", bufs=2) as p_tmp2,
                tc.tile_pool(name="ps2", bufs=6, space="PSUM") as ps2,
            ):
                ag = []
                for i in range(NC):
                    t = p_agg.tile([128, TOKC], bf16, tag="ag")
                    nc.sync.dma_start(t[:, 0:HT], a2a_out[0][i])
                    nc.sync.dma_start(t[:, HT:TOKC], a2a_out[1][i])
                    ag.append(t)

                wp_sb = []
                for kt in range(CT):
                    t = p_wbig.tile([128, C], bf16, tag="wp")
                    nc.sync.dma_start(t[:], wp_d[kt * 128:(kt + 1) * 128, :])
                    wp_sb.append(t)

                xres = []
                for ct in range(CT):
                    t = p_act2.tile([128, TOKC], f32r, tag="xres")
                    nc.sync.dma_start(t[:], xres_d[ct * 128:(ct + 1) * 128, :])
                    xres.append(t)

                def layer_norm(x_tiles, g_b, be_b, out_dtype, out_pool, out_tag,
                               sq_pool, tmp_pool, ps_pool):
                    """Feature-major LN across CT tiles of [128, TOKC]."""
                    s1 = ps_pool.tile([1, TOKC], f32, tag="ps")
                    s2 = ps_pool.tile([1, TOKC], f32, tag="ps")
                    for ct in range(CT):
                        nc.tensor.matmul(
                            s1[:], ones[:, 0:1], x_tiles[ct][:],
                            start=(ct == 0), stop=(ct == CT - 1),
                        )
                    for ct in range(CT):
                        sq = sq_pool.tile([128, TOKC], f32r, tag="sq")
                        nc.vector.tensor_mul(
                            sq[:],
                            x_tiles[ct][:].bitcast(f32),
                            x_tiles[ct][:].bitcast(f32),
                        )
                        nc.tensor.matmul(
                            s2[:], ones[:, 0:1], sq[:],
                            start=(ct == 0), stop=(ct == CT - 1),
                        )
                    nmu = tmp_pool.tile([1, TOKC], f32r, tag="nmu")
                    nc.vector.tensor_scalar_mul(nmu[:], s1[:], -1.0 / C)
                    ex2 = tmp_pool.tile([1, TOKC], f32, tag="ex2")
                    nc.vector.tensor_scalar_mul(ex2[:], s2[:], 1.0 / C)
                    mu2 = tmp_pool.tile([1, TOKC], f32, tag="mu2")
                    nc.vector.tensor_mul(
                        mu2[:], nmu[:].bitcast(f32), nmu[:].bitcast(f32)
                    )
                    var = tmp_pool.tile([1, TOKC], f32, tag="var")
                    nc.vector.tensor_sub(var[:], ex2[:], mu2[:])
                    nc.vector.tensor_scalar_add(var[:], var[:], LN_EPS)
                    sd = tmp_pool.tile([1, TOKC], f32r, tag="sd")
                    nc.scalar.activation(sd[:], var[:], AF.Sqrt, bias=0.0)
                    bmu = ps_pool.tile([128, TOKC], f32, tag="ps")
                    nc.tensor.matmul(
                        bmu[:], ones[0:1, :], nmu[:], start=True, stop=True
                    )
                    brs = ps_pool.tile([128, TOKC], f32, tag="ps")
                    nc.tensor.matmul(
                        brs[:], ones[0:1, :], sd[:], start=True, stop=True
                    )
                    bmu_sb = tmp_pool.tile([128, TOKC], f32, tag="bmu")
                    nc.scalar.copy(bmu_sb[:], bmu[:])
                    brs_sb = tmp_pool.tile([128, TOKC], f32, tag="brs")
                    nc.vector.reciprocal_approx_fast(brs_sb[:], brs[:])
                    outs = []
                    for ct in range(CT):
                        t1 = tmp_pool.tile([128, TOKC], f32, tag="lntmp")
                        nc.vector.tensor_add(
                            t1[:], x_tiles[ct][:].bitcast(f32), bmu_sb[:]
                        )
                        t2 = tmp_pool.tile([128, TOKC], f32, tag="lntmp2")
                        nc.vector.tensor_mul(t2[:], t1[:], brs_sb[:])
                        o = out_pool.tile([128, TOKC], out_dtype, tag=out_tag)
                        nc.scalar.activation(
                            o[:], t2[:], AF.Identity,
                            bias=be_b[:, ct:ct + 1], scale=g_b[:, ct:ct + 1],
                        )
                        outs.append(o)
                    return outs

                # ---- proj + bias + residual ----
                x1 = []
                for mt in range(CT):
                    yps = ps2.tile([128, TOKC], f32, tag="ps")
                    for kt in range(CT):
                        nc.tensor.matmul(
                            yps[:], wp_sb[kt][:, mt * 128:(mt + 1) * 128], ag[kt][:],
                            start=(kt == 0), stop=(kt == CT - 1),
                        )
                    t1 = p_tmp2.tile([128, TOKC], f32, tag="projt")
                    nc.scalar.activation(
                        t1[:], yps[:], AF.Identity, bias=bproj_b[:, mt:mt + 1]
                    )
                    xr = p_act2.tile([128, TOKC], f32r, tag="x1")
                    nc.vector.tensor_add(xr[:], t1[:], xres[mt][:].bitcast(f32))
                    x1.append(xr)

                ln1 = layer_norm(
                    x1, g1_b, be1_b, bf16, p_ln1, "ln1", p_tmp2, p_tmp2, ps2
                )

            # ======== phase 3: FFN + LN2 + out ========
            with (
                tc.tile_pool(name="hff", bufs=FT) as p_hff,
                tc.tile_pool(name="w1s", bufs=4) as p_w1,
                tc.tile_pool(name="w2s", bufs=4) as p_w2,
                tc.tile_pool(name="act3", bufs=8) as p_act3,
                tc.tile_pool(name="tmp3", bufs=2) as p_tmp3,
                tc.tile_pool(name="outp", bufs=2) as p_out,
            ):
                # ---- FFN1: h = relu(w1^T @ ln1 + b1) ----
                hff = []
                with tc.tile_pool(name="ps3a", bufs=4, space="PSUM") as ps3a:
                    for mt in range(FT):
                        w1t = p_w1.tile([128, CT, 128], bf16, tag="w1")
                        nc.sync.dma_start(w1t[:], w1_d[mt])
                        yps = ps3a.tile([128, TOKC], f32, tag="ps")
                        for kt in range(CT):
                            nc.tensor.matmul(
                                yps[:], w1t[:, kt, :], ln1[kt][:],
                                start=(kt == 0), stop=(kt == CT - 1),
                            )
                        hf = p_hff.tile([128, TOKC], bf16, tag="hff")
                        nc.scalar.activation(
                            hf[:], yps[:], AF.Relu, bias=b1t[:, mt:mt + 1]
                        )
                        hff.append(hf)

                # ---- FFN2 (kt-outer, 8 accumulators) + LN2 ----
                with tc.tile_pool(name="ps3b", bufs=8, space="PSUM") as ps3b:
                    accs = [
                        ps3b.tile([128, TOKC], f32, tag="ps", name=f"acc{mt}")
                        for mt in range(CT)
                    ]
                    for kt in range(FT):
                        w2t = p_w2.tile([128, C], bf16, tag="w2")
                        nc.sync.dma_start(w2t[:], w2_d[kt * 128:(kt + 1) * 128, :])
                        for mt in range(CT):
                            nc.tensor.matmul(
                                accs[mt][:],
                                w2t[:, mt * 128:(mt + 1) * 128],
                                hff[kt][:],
                                start=(kt == 0), stop=(kt == FT - 1),
                            )
                    x2 = []
                    for mt in range(CT):
                        t1 = p_tmp3.tile([128, TOKC], f32, tag="ffn2t")
                        nc.scalar.activation(
                            t1[:], accs[mt][:], AF.Identity, bias=b2_b[:, mt:mt + 1]
                        )
                        xr = p_act3.tile([128, TOKC], f32r, tag="x2")
                        nc.vector.tensor_add(xr[:], t1[:], ln1[mt][:])
                        x2.append(xr)

                    out_tiles = layer_norm(
                        x2, g2_b, be2_b, f32, p_out, "outt", p_tmp3, p_tmp3, ps3b
                    )
                    for ct in range(CT):
                        nc.sync.dma_start(
                            out_d[ct * 128:(ct + 1) * 128, :], out_tiles[ct][:]
                        )

    nc.compile()
    return nc


def _pack_inputs(inputs):
    """Host-side sharding/marshalling. Returns in_maps for the 8 cores."""
    x = np.asarray(inputs["x"], dtype=np.float32)
    xf = np.ascontiguousarray(x.reshape(TOK, C))
    xt = np.ascontiguousarray(xf.T)  # [C, TOK]
    wq = np.asarray(inputs["wq"], dtype=np.float32)
    wk = np.asarray(inputs["wk"], dtype=np.float32)
    wv = np.asarray(inputs["wv"], dtype=np.float32)
    wproj = np.ascontiguousarray(
        np.asarray(inputs["w_proj"], dtype=np.float32).astype(ml_dtypes.bfloat16)
    )
    w1 = np.asarray(inputs["w1"], dtype=np.float32)
    w2 = np.ascontiguousarray(
        np.asarray(inputs["w2"], dtype=np.float32).astype(ml_dtypes.bfloat16)
    )
    # w1 packed per ff-tile: [FT, 128(p), CT, 128(f)];  w1 is [C, DFF]
    w1p = np.ascontiguousarray(
        w1.reshape(CT, 128, FT, 128).transpose(2, 1, 0, 3).astype(ml_dtypes.bfloat16)
    )

    def tile_vec(v, n):
        return np.ascontiguousarray(
            np.asarray(v, dtype=np.float32).reshape(n, 128).T
        )

    biaspack = np.zeros((128, 6 * CT), dtype=np.float32)
    biaspack[:, 0 * CT:1 * CT] = tile_vec(inputs["b_proj"], CT)
    biaspack[:, 1 * CT:2 * CT] = tile_vec(inputs["b2"], CT)
    biaspack[:, 2 * CT:3 * CT] = tile_vec(inputs["g1"], CT)
    biaspack[:, 3 * CT:4 * CT] = tile_vec(inputs["be1"], CT)
    biaspack[:, 4 * CT:5 * CT] = tile_vec(inputs["g2"], CT)
    biaspack[:, 5 * CT:6 * CT] = tile_vec(inputs["be2"], CT)
    b1t = tile_vec(inputs["b1"], FT)

    # causal masks for the 4 diagonal offsets, packed [128, 4, 512]
    r = np.arange(128)[:, None]
    ccol = np.arange(512)[None, :]
    cmask = np.stack(
        [(ccol >= r + 128 * m).astype(np.float32) for m in range(4)], axis=1
    )
    cmask = np.ascontiguousarray(cmask)  # [128, 4, 512]
    onesp = np.ones((128, 128), dtype=np.float32)
    ident = np.eye(128, dtype=np.float32)

    in_maps = []
    for c in range(NC):
        hcol = slice(c * DH, (c + 1) * DH)

        def pack_w(w):
            return np.ascontiguousarray(
                w[:, hcol].reshape(CT, 128, DH).transpose(1, 0, 2)
            )

        in_maps.append(
            {
                "xt": xt,
                "wq_c": pack_w(wq),
                "wk_c": pack_w(wk),
                "wv_c": pack_w(wv),
                "xres_c": np.ascontiguousarray(
                    np.concatenate(
                        [
                            xt[:, b * T + c * (TOKC // B):
                               b * T + (c + 1) * (TOKC // B)]
                            for b in range(B)
                        ],
                        axis=1,
                    )
                ),
                "wproj": wproj,
                "w1p": w1p,
                "w2": w2,
                "biaspack": biaspack,
                "b1t": b1t,
                "onesp": onesp,
                "ident": ident,
                "cmask": cmask,
            }
        )
    return in_maps


def _run(inputs, trace=False, debug=False):
    key = "dbg" if debug else "rel"
    if key not in _CACHE:
        _CACHE[key] = _build(debug=debug)
    nc = _CACHE[key]
    in_maps = _pack_inputs(inputs)
    res = bass_utils.run_bass_kernel_spmd(
        nc, in_maps, core_ids=list(range(NC)), trace=trace
    )
    out = np.empty((TOK, C), dtype=np.float32)
    ht = TOKC // B
    for c in range(NC):
        oc = res.results[c]["out"]
        for b in range(B):
            out[b * T + c * ht:b * T + (c + 1) * ht, :] = (
                oc[:, b * ht:(b + 1) * ht].T
            )
    return out.reshape(B, T, C), res


def kernel(**inputs) -> np.ndarray:
    out, _ = _run(inputs, trace=False, debug=False)
    return out


# revision 31
# speedup vs baseline: 1.4593x; 1.0473x over previous
"""Trainium2 Bass kernel for a dense transformer block (B=2, T=2048, C=1024,
H=16, Dff=4096), SPMD across 8 NeuronCores.

Sharding: attention is head-parallel (2 heads/core); one AllToAll
redistributes the attention output into a token-parallel layout; projection,
layernorms and the FFN then run on each core's 512-token slice with full
weights. All on-device activations are kept feature-major (transposed) so
every matmul consumes weights exactly as stored; the host performs the
x -> x^T and out^T -> out transposes during input/output marshalling.
Matmuls run in float32r (fp32 storage, FP22 multiply, fp32 accumulate).
"""

import sys

sys.path.insert(0, "/opt/trn_rl_repo")

import numpy as np
import ml_dtypes
import concourse.bacc as bacc
import concourse.mybir as mybir
import concourse.tile as tile
import concourse.bass_utils as bass_utils

try:  # make the NTFF profile shim importable as antenv.axon_hooks
    import antenv

    if "/opt/trn_rl_repo/antenv" not in antenv.__path__:
        antenv.__path__.append("/opt/trn_rl_repo/antenv")
except Exception:
    pass

f32 = mybir.dt.float32
f32r = mybir.dt.float32r
bf16 = mybir.dt.bfloat16
AF = mybir.ActivationFunctionType
ALU = mybir.AluOpType

NC = 8          # cores
B = 2           # batch
T = 2048        # sequence length
C = 1024        # model dim
H = 16          # heads
HD = 64         # head dim
HPC = H // NC   # heads per core (2)
DH = HPC * HD   # per-core head cols (128)
DFF = 4096
TOK = B * T     # 4096 tokens
TOKC = TOK // NC  # 512 tokens per core
CT = C // 128   # 8 c-tiles
FT = DFF // 128  # 32 ff-tiles
KT = T // 128   # 16 k-tiles per batch
QC = T // 512   # 4 q-chunks of 512 per batch
LN_EPS = 1e-5

_CACHE = {}


def _build(debug=False):
    nc = bacc.Bacc("TRN2", target_bir_lowering=False, debug=False, num_devices=NC)

    # ---- DRAM I/O (per-core values supplied via in_maps) ----
    xt_d = nc.dram_tensor("xt", [C, TOK], f32r, kind="ExternalInput")
    wq_d = nc.dram_tensor("wq_c", [128, CT, 128], f32r, kind="ExternalInput")
    wk_d = nc.dram_tensor("wk_c", [128, CT, 128], f32r, kind="ExternalInput")
    wv_d = nc.dram_tensor("wv_c", [128, CT, 128], f32r, kind="ExternalInput")
    xres_d = nc.dram_tensor("xres_c", [C, TOKC], f32r, kind="ExternalInput")
    wp_d = nc.dram_tensor("wproj", [C, C], bf16, kind="ExternalInput")
    w1_d = nc.dram_tensor("w1p", [FT, 128, CT, 128], bf16, kind="ExternalInput")
    w2_d = nc.dram_tensor("w2", [DFF, C], bf16, kind="ExternalInput")
    bias_d = nc.dram_tensor("biaspack", [128, 6 * CT], f32, kind="ExternalInput")
    b1_d = nc.dram_tensor("b1t", [128, FT], f32, kind="ExternalInput")
    ones_d = nc.dram_tensor("onesp", [128, 128], f32r, kind="ExternalInput")
    ident_d = nc.dram_tensor("ident", [128, 128], f32, kind="ExternalInput")
    mask_d = nc.dram_tensor("cmask", [128, 4, 512], bf16, kind="ExternalInput")
    out_d = nc.dram_tensor("out", [C, TOKC], f32, kind="ExternalOutput")
    if debug:
        dbg_attn = nc.dram_tensor("dbg_attn", [2, HD, TOK], f32, kind="ExternalOutput")
        dbg_qkv = nc.dram_tensor("dbg_qkv", [3, DH, TOK], f32, kind="ExternalOutput")

    with tile.TileContext(nc) as tc:
        with (
            nc.allow_low_precision(reason="float32r matmul inputs (~6e-5 rounding)"),
            tc.tile_pool(name="const", bufs=1) as p_const,
            tc.tile_pool(name="ln1p", bufs=CT) as p_ln1,
            tc.tile_pool(name="dram", bufs=1, space="DRAM") as p_dram,
        ):
            # ---- persistent constants ----
            ones = p_const.tile([128, 128], f32r, tag="ones")
            nc.sync.dma_start(ones[:], ones_d[:])
            biasp = p_const.tile([128, 6 * CT], f32, tag="biasp")
            nc.sync.dma_start(biasp[:], bias_d[:])
            b1t = p_const.tile([128, FT], f32, tag="b1t")
            nc.sync.dma_start(b1t[:], b1_d[:])
            # bias pack columns: [bproj | b2 | g1 | be1 | g2 | be2]
            bproj_b = biasp[:, 0 * CT:1 * CT]
            b2_b = biasp[:, 1 * CT:2 * CT]
            g1_b = biasp[:, 2 * CT:3 * CT]
            be1_b = biasp[:, 3 * CT:4 * CT]
            g2_b = biasp[:, 4 * CT:5 * CT]
            be2_b = biasp[:, 5 * CT:6 * CT]

            # two half-AllToAlls: batch-0 shards exchange while batch-1
            # attention still computes. Core c owns tokens
            # [c*256,(c+1)*256) of each batch (512 total).
            HT = TOKC // B  # 256 tokens per batch per core
            a2a_in = [
                p_dram.tile([NC, DH, HT], bf16, tag=f"a2ai{b}", name=f"a2ai{b}")
                for b in range(B)
            ]
            a2a_out = [
                p_dram.tile([NC, DH, HT], bf16, tag=f"a2ao{b}", name=f"a2ao{b}")
                for b in range(B)
            ]

            # ======== phase 1: QKV + attention (head-parallel) ========
            with (
                tc.tile_pool(name="attn", bufs=1) as p_attn,
                tc.tile_pool(name="p1c", bufs=1) as p1c,
                tc.tile_pool(name="xt", bufs=8) as p_xt,
                tc.tile_pool(name="qkv", bufs=1) as p_qkv,
                tc.tile_pool(name="es", bufs=3) as p_es,
                tc.tile_pool(name="small", bufs=2) as p_small,
                tc.tile_pool(name="ps1", bufs=6, space="PSUM") as ps1,
            ):
                # per-head attention outputs (feature-major rows 0-63)
                attnh = [
                    p_attn.tile([HD, TOK], bf16, tag=f"attn{h}", name=f"attnh{h}")
                    for h in range(HPC)
                ]
                ident = p1c.tile([128, 128], f32, tag="ident")
                nc.sync.dma_start(ident[:], ident_d[:])
                identb = p1c.tile([128, 128], bf16, tag="identb")
                nc.vector.tensor_copy(identb[:], ident[:])
                masks = p1c.tile([128, 4, 512], bf16, tag="masks")
                nc.sync.dma_start(masks[:], mask_d[:])
                wq_sb = p1c.tile([128, CT, 128], f32r, tag="wq")
                wk_sb = p1c.tile([128, CT, 128], f32r, tag="wk")
                wv_sb = p1c.tile([128, CT, 128], f32r, tag="wv")
                nc.sync.dma_start(wq_sb[:], wq_d[:])
                nc.sync.dma_start(wk_sb[:], wk_d[:])
                nc.sync.dma_start(wv_sb[:], wv_d[:])

                for b in range(B):
                    qT = p_qkv.tile([DH, T], f32r, tag="q")
                    kT = p_qkv.tile([DH, T], f32r, tag="k")
                    vT = p_qkv.tile([DH, T], f32, tag="v")

                    # ---- QKV projections (feature-major), x^T streamed per
                    # (ct, chunk) so only ~2 chunks of x^T are resident ----
                    for n in range(QC):
                        ncol = slice(n * 512, (n + 1) * 512)
                        xt_sb = []
                        for ct in range(CT):
                            t = p_xt.tile([128, 512], f32r, tag="xt", bufs=16)
                            nc.sync.dma_start(
                                t[:],
                                xt_d[ct * 128:(ct + 1) * 128,
                                     b * T + n * 512:b * T + (n + 1) * 512],
                            )
                            xt_sb.append(t)
                        pq = ps1.tile([128, 512], f32, tag="sps", bufs=3)
                        pk = ps1.tile([128, 512], f32, tag="sps", bufs=3)
                        for ct in range(CT):
                            st = ct == 0
                            sp = ct == CT - 1
                            nc.tensor.matmul(
                                pq[:], wq_sb[:, ct, :], xt_sb[ct][:],
                                start=st, stop=sp,
                            )
                            nc.tensor.matmul(
                                pk[:], wk_sb[:, ct, :], xt_sb[ct][:],
                                start=st, stop=sp,
                            )
                        nc.vector.tensor_copy(qT[:, ncol], pq[:])
                        nc.vector.tensor_copy(kT[:, ncol], pk[:])
                        pv = ps1.tile([128, 512], f32, tag="sps", bufs=3)
                        for ct in range(CT):
                            nc.tensor.matmul(
                                pv[:], wv_sb[:, ct, :], xt_sb[ct][:],
                                start=(ct == 0), stop=(ct == CT - 1),
                            )
                        nc.scalar.copy(vT[:, ncol], pv[:])

                    # ---- V -> token-major; slot: [Vh0|ones|pad|Vh1|ones|pad] ----
                    SL = 132
                    vt = p_qkv.tile([128, KT, SL], bf16, tag="vt")
                    for kt in range(KT):
                        pt = ps1.tile([128, 128], f32, tag="oacc", bufs=2)
                        nc.tensor.transpose(
                            pt[:], vT[:, kt * 128:(kt + 1) * 128], ident[:]
                        )
                        nc.vector.tensor_copy(vt[:, kt, 0:64], pt[:, 0:64])
                        nc.vector.tensor_copy(vt[:, kt, 66:130], pt[:, 64:128])
                        nc.vector.tensor_copy(vt[:, kt, 64:65], ones[:, 0:1])
                        nc.vector.tensor_copy(vt[:, kt, 130:131], ones[:, 1:2])

                    # ---- causal attention: both heads interleaved per k-tile
                    # (adjacent score matmuls pack into disjoint array rows) ----
                    for j in range(QC):
                        nkt = 4 * j + 4
                        oacc = [
                            ps1.tile([65, 512], f32, tag="oacc", bufs=2,
                                     name=f"oacc{h}")
                            for h in range(HPC)
                        ]
                        for kt in range(nkt):
                            m = kt - 4 * j
                            # both heads' scores into one 2-bank PSUM tile so a
                            # single exp (and one wait) covers both attnV MMs
                            spair = ps1.tile([128, 2, 512], f32, tag="sps", bufs=3)
                            for h in range(HPC):
                                hrow = slice(h * 64, (h + 1) * 64)
                                nc.tensor.matmul(
                                    spair[:, h, :],
                                    kT[hrow, kt * 128:(kt + 1) * 128],
                                    qT[hrow, j * 512:(j + 1) * 512],
                                    start=True, stop=True,
                                    tile_position=(64 * h, 0),
                                )
                            epair = p_es.tile([128, 2, 512], bf16, tag="es", bufs=6)
                            if m < 0:
                                nc.scalar.activation(
                                    epair[:], spair[:], AF.Exp, scale=0.125
                                )
                            else:
                                ed = p_es.tile([128, 2, 512], bf16, tag="esd", bufs=2)
                                nc.scalar.activation(
                                    ed[:], spair[:], AF.Exp, scale=0.125
                                )
                                for h in range(HPC):
                                    nc.vector.tensor_mul(
                                        epair[:, h, :], ed[:, h, :],
                                        masks[:, m, :],
                                    )
                            for h in range(HPC):
                                nc.tensor.matmul(
                                    oacc[h][:],
                                    vt[:, kt, 66 * h:66 * h + 65],
                                    epair[:, h, :],
                                    start=(kt == 0), stop=(kt == nkt - 1),
                                )
                        for h in range(HPC):
                            # free the PSUM accum early via two same-base copies
                            osb = p_small.tile([64, 512], f32, tag="osb", bufs=4)
                            nc.scalar.copy(osb[:], oacc[h][0:64, :])
                            sr = p_small.tile([128, 512], f32r, tag="sr")
                            nc.vector.tensor_copy(sr[64:65, :], oacc[h][64:65, :])
                            # broadcast sums across partitions, then approx-recip
                            bps = ps1.tile([64, 512], f32, tag="oacc", bufs=2)
                            nc.tensor.matmul(
                                bps[:], ones[64:65, 0:64], sr[64:65, :],
                                start=True, stop=True, tile_position=(64, 0),
                            )
                            ibc = p_small.tile([64, 512], f32, tag="ibc")
                            nc.vector.reciprocal_approx_fast(ibc[:], bps[:])
                            nc.vector.tensor_mul(
                                attnh[h][:, b * T + j * 512:b * T + (j + 1) * 512],
                                osb[:],
                                ibc[:],
                            )
                        # stage this chunk's two A2A shards immediately
                        for s in (2 * j, 2 * j + 1):
                            for h in range(HPC):
                                nc.sync.dma_start(
                                    a2a_in[b][s, h * 64:(h + 1) * 64, :],
                                    attnh[h][:, b * T + s * HT:b * T + (s + 1) * HT],
                                )

                    if debug:
                        nc.sync.dma_start(
                            dbg_qkv[0, :, b * T:(b + 1) * T], qT[:].bitcast(f32)
                        )
                        nc.sync.dma_start(
                            dbg_qkv[1, :, b * T:(b + 1) * T], kT[:].bitcast(f32)
                        )
                        nc.sync.dma_start(dbg_qkv[2, :, b * T:(b + 1) * T], qT[:].bitcast(f32))

                    # ---- AllToAll for this batch's token shards (staged
                    # incrementally above); the b=0 exchange overlaps
                    # batch-1 QKV + attention ----
                    nc.gpsimd.collective_compute(
                        "AllToAll",
                        ALU.bypass,
                        replica_groups=[list(range(NC))],
                        ins=[a2a_in[b][:].opt()],
                        outs=[a2a_out[b][:].opt()],
                    )

                if debug:
                    for h in range(HPC):
                        nc.sync.dma_start(dbg_attn[h], attnh[h][:].bitcast(f32))

            # ======== phase 2: gather + proj + LN1 ========
            with (
                tc.tile_pool(name="agg", bufs=8) as p_agg,
                tc.tile_pool(name="wbig", bufs=8) as p_wbig,
                tc.tile_pool(name="act2", bufs=8) as p_act2,
                tc.tile_pool(name="tmp2", bufs=2) as p_tmp2,
                tc.tile_pool(name="ps2", bufs=6, space="PSUM") as ps2,
            ):
                ag = []
                for i in range(NC):
                    t = p_agg.tile([128, TOKC], bf16, tag="ag")
                    nc.sync.dma_start(t[:, 0:HT], a2a_out[0][i])
                    nc.sync.dma_start(t[:, HT:TOKC], a2a_out[1][i])
                    ag.append(t)

                wp_sb = []
                for kt in range(CT):
                    t = p_wbig.tile([128, C], bf16, tag="wp")
                    nc.sync.dma_start(t[:], wp_d[kt * 128:(kt + 1) * 128, :])
                    wp_sb.append(t)

                xres = []
                for ct in range(CT):
                    t = p_act2.tile([128, TOKC], f32r, tag="xres")
                    nc.sync.dma_start(t[:], xres_d[ct * 128:(ct + 1) * 128, :])
                    xres.append(t)

                def layer_norm(x_tiles, g_b, be_b, out_dtype, out_pool, out_tag,
                               sq_pool, tmp_pool, ps_pool):
                    """Feature-major LN across CT tiles of [128, TOKC]."""
                    s1 = ps_pool.tile([1, TOKC], f32, tag="ps")
                    s2 = ps_pool.tile([1, TOKC], f32, tag="ps")
                    for ct in range(CT):
                        nc.tensor.matmul(
                            s1[:], ones[:, 0:1], x_tiles[ct][:],
                            start=(ct == 0), stop=(ct == CT - 1),
                        )
                    for ct in range(CT):
                        sq = sq_pool.tile([128, TOKC], f32r, tag="sq")
                        nc.vector.tensor_mul(
                            sq[:],
                            x_tiles[ct][:].bitcast(f32),
                            x_tiles[ct][:].bitcast(f32),
                        )
                        nc.tensor.matmul(
                            s2[:], ones[:, 0:1], sq[:],
                            start=(ct == 0), stop=(ct == CT - 1),
                        )
                    nmu = tmp_pool.tile([1, TOKC], f32r, tag="nmu")
                    nc.vector.tensor_scalar_mul(nmu[:], s1[:], -1.0 / C)
                    ex2 = tmp_pool.tile([1, TOKC], f32, tag="ex2")
                    nc.vector.tensor_scalar_mul(ex2[:], s2[:], 1.0 / C)
                    mu2 = tmp_pool.tile([1, TOKC], f32, tag="mu2")
                    nc.vector.tensor_mul(
                        mu2[:], nmu[:].bitcast(f32), nmu[:].bitcast(f32)
                    )
                    var = tmp_pool.tile([1, TOKC], f32, tag="var")
                    nc.vector.tensor_sub(var[:], ex2[:], mu2[:])
                    nc.vector.tensor_scalar_add(var[:], var[:], LN_EPS)
                    sd = tmp_pool.tile([1, TOKC], f32r, tag="sd")
                    nc.scalar.activation(sd[:], var[:], AF.Sqrt, bias=0.0)
                    bmu = ps_pool.tile([128, TOKC], f32, tag="ps")
                    nc.tensor.matmul(
                        bmu[:], ones[0:1, :], nmu[:], start=True, stop=True
                    )
                    brs = ps_pool.tile([128, TOKC], f32, tag="ps")
                    nc.tensor.matmul(
                        brs[:], ones[0:1, :], sd[:], start=True, stop=True
                    )
                    bmu_sb = tmp_pool.tile([128, TOKC], f32, tag="bmu")
                    nc.scalar.copy(bmu_sb[:], bmu[:])
                    brs_sb = tmp_pool.tile([128, TOKC], f32, tag="brs")
                    nc.vector.reciprocal_approx_fast(brs_sb[:], brs[:])
                    outs = []
                    for ct in range(CT):
                        t1 = tmp_pool.tile([128, TOKC], f32, tag="lntmp")
                        nc.vector.tensor_add(
                            t1[:], x_tiles[ct][:].bitcast(f32), bmu_sb[:]
                        )
                        t2 = tmp_pool.tile([128, TOKC], f32, tag="lntmp2")
                        nc.vector.tensor_mul(t2[:], t1[:], brs_sb[:])
                        o = out_pool.tile([128, TOKC], out_dtype, tag=out_tag)
                        nc.scalar.activation(
                            o[:], t2[:], AF.Identity,
                            bias=be_b[:, ct:ct + 1], scale=g_b[:, ct:ct + 1],
                        )
                        outs.append(o)
                    return outs

                # ---- proj + bias + residual ----
                x1 = []
                for mt in range(CT):
                    yps = ps2.tile([128, TOKC], f32, tag="ps")
                    for kt in range(CT):
                        nc.tensor.matmul(
                            yps[:], wp_sb[kt][:, mt * 128:(mt + 1) * 128], ag[kt][:],
                            start=(kt == 0), stop=(kt == CT - 1),
                        )
                    t1 = p_tmp2.tile([128, TOKC], f32, tag="projt")
                    nc.scalar.activation(
                        t1[:], yps[:], AF.Identity, bias=bproj_b[:, mt:mt + 1]
                    )
                    xr = p_act2.tile([128, TOKC], f32r, tag="x1")
                    nc.vector.tensor_add(xr[:], t1[:], xres[mt][:].bitcast(f32))
                    x1.append(xr)

                ln1 = layer_norm(
                    x1, g1_b, be1_b, bf16, p_ln1, "ln1", p_tmp2, p_tmp2, ps2
                )

            # ======== phase 3: FFN + LN2 + out ========
            with (
                tc.tile_pool(name="hff", bufs=FT) as p_hff,
                tc.tile_pool(name="w1s", bufs=4) as p_w1,
                tc.tile_pool(name="w2s", bufs=4) as p_w2,
                tc.tile_pool(name="act3", bufs=8) as p_act3,
                tc.tile_pool(name="tmp3", bufs=2) as p_tmp3,
                tc.tile_pool(name="outp", bufs=2) as p_out,
            ):
                # ---- FFN1: h = relu(w1^T @ ln1 + b1) ----
                hff = []
                with tc.tile_pool(name="ps3a", bufs=4, space="PSUM") as ps3a:
                    for mt in range(FT):
                        w1t = p_w1.tile([128, CT, 128], bf16, tag="w1")
                        nc.sync.dma_start(w1t[:], w1_d[mt])
                        yps = ps3a.tile([128, TOKC], f32, tag="ps")
                        for kt in range(CT):
                            nc.tensor.matmul(
                                yps[:], w1t[:, kt, :], ln1[kt][:],
                                start=(kt == 0), stop=(kt == CT - 1),
                            )
                        hf = p_hff.tile([128, TOKC], bf16, tag="hff")
                        nc.scalar.activation(
                            hf[:], yps[:], AF.Relu, bias=b1t[:, mt:mt + 1]
                        )
                        hff.append(hf)

                # ---- FFN2 (kt-outer, 8 accumulators) + LN2 ----
                with tc.tile_pool(name="ps3b", bufs=8, space="PSUM") as ps3b:
                    accs = [
                        ps3b.tile([128, TOKC], f32, tag="ps", name=f"acc{mt}")
                        for mt in range(CT)
                    ]
                    for kt in range(FT):
                        w2t = p_w2.tile([128, C], bf16, tag="w2")
                        nc.sync.dma_start(w2t[:], w2_d[kt * 128:(kt + 1) * 128, :])
                        for mt in range(CT):
                            nc.tensor.matmul(
                                accs[mt][:],
                                w2t[:, mt * 128:(mt + 1) * 128],
                                hff[kt][:],
                                start=(kt == 0), stop=(kt == FT - 1),
                            )
                    x2 = []
                    for mt in range(CT):
                        t1 = p_tmp3.tile([128, TOKC], f32, tag="ffn2t")
                        nc.scalar.activation(
                            t1[:], accs[mt][:], AF.Identity, bias=b2_b[:, mt:mt + 1]
                        )
                        xr = p_act3.tile([128, TOKC], f32r, tag="x2")
                        nc.vector.tensor_add(xr[:], t1[:], ln1[mt][:])
                        x2.append(xr)

                    out_tiles = layer_norm(
                        x2, g2_b, be2_b, f32, p_out, "outt", p_tmp3, p_tmp3, ps3b
                    )
                    for ct in range(CT):
                        nc.sync.dma_start(
                            out_d[ct * 128:(ct + 1) * 128, :], out_tiles[ct][:]
                        )

    nc.compile()
    return nc


def _pack_inputs(inputs):
    """Host-side sharding/marshalling. Returns in_maps for the 8 cores."""
    x = np.asarray(inputs["x"], dtype=np.float32)
    xf = np.ascontiguousarray(x.reshape(TOK, C))
    xt = np.ascontiguousarray(xf.T)  # [C, TOK]
    wq = np.asarray(inputs["wq"], dtype=np.float32)
    wk = np.asarray(inputs["wk"], dtype=np.float32)
    wv = np.asarray(inputs["wv"], dtype=np.float32)
    wproj = np.ascontiguousarray(
        np.asarray(inputs["w_proj"], dtype=np.float32).astype(ml_dtypes.bfloat16)
    )
    w1 = np.asarray(inputs["w1"], dtype=np.float32)
    w2 = np.ascontiguousarray(
        np.asarray(inputs["w2"], dtype=np.float32).astype(ml_dtypes.bfloat16)
    )
    # w1 packed per ff-tile: [FT, 128(p), CT, 128(f)];  w1 is [C, DFF]
    w1p = np.ascontiguousarray(
        w1.reshape(CT, 128, FT, 128).transpose(2, 1, 0, 3).astype(ml_dtypes.bfloat16)
    )

    def tile_vec(v, n):
        return np.ascontiguousarray(
            np.asarray(v, dtype=np.float32).reshape(n, 128).T
        )

    biaspack = np.zeros((128, 6 * CT), dtype=np.float32)
    biaspack[:, 0 * CT:1 * CT] = tile_vec(inputs["b_proj"], CT)
    biaspack[:, 1 * CT:2 * CT] = tile_vec(inputs["b2"], CT)
    biaspack[:, 2 * CT:3 * CT] = tile_vec(inputs["g1"], CT)
    biaspack[:, 3 * CT:4 * CT] = tile_vec(inputs["be1"], CT)
    biaspack[:, 4 * CT:5 * CT] = tile_vec(inputs["g2"], CT)
    biaspack[:, 5 * CT:6 * CT] = tile_vec(inputs["be2"], CT)
    b1t = tile_vec(inputs["b1"], FT)

    # causal masks for the 4 diagonal offsets, packed [128, 4, 512]
    r = np.arange(128)[:, None]
    ccol = np.arange(512)[None, :]
    cmask = np.stack(
        [(ccol >= r + 128 * m).astype(np.float32) for m in range(4)], axis=1
    )
    cmask = np.ascontiguousarray(cmask).astype(ml_dtypes.bfloat16)  # [128, 4, 512]
    onesp = np.ones((128, 128), dtype=np.float32)
    ident = np.eye(128, dtype=np.float32)

    in_maps = []
    for c in range(NC):
        hcol = slice(c * DH, (c + 1) * DH)

        def pack_w(w):
            return np.ascontiguousarray(
                w[:, hcol].reshape(CT, 128, DH).transpose(1, 0, 2)
            )

        in_maps.append(
            {
                "xt": xt,
                "wq_c": pack_w(wq),
                "wk_c": pack_w(wk),
                "wv_c": pack_w(wv),
                "xres_c": np.ascontiguousarray(
                    np.concatenate(
                        [
                            xt[:, b * T + c * (TOKC // B):
                               b * T + (c + 1) * (TOKC // B)]
                            for b in range(B)
                        ],
                        axis=1,
                    )
                ),
                "wproj": wproj,
                "w1p": w1p,
                "w2": w2,
                "biaspack": biaspack,
                "b1t": b1t,
                "onesp": onesp,
                "ident": ident,
                "cmask": cmask,
            }
        )
    return in_maps


def _run(inputs, trace=False, debug=False):
    key = "dbg" if debug else "rel"
    if key not in _CACHE:
        _CACHE[key] = _build(debug=debug)
    nc = _CACHE[key]
    in_maps = _pack_inputs(inputs)
    res = bass_utils.run_bass_kernel_spmd(
        nc, in_maps, core_ids=list(range(NC)), trace=trace
    )
    out = np.empty((TOK, C), dtype=np.float32)
    ht = TOKC // B
    for c in range(NC):
        oc = res.results[c]["out"]
        for b in range(B):
            out[b * T + c * ht:b * T + (c + 1) * ht, :] = (
                oc[:, b * ht:(b + 1) * ht].T
            )
    return out.reshape(B, T, C), res


def kernel(**inputs) -> np.ndarray:
    out, _ = _run(inputs, trace=False, debug=False)
    return out
